# revision 1
# baseline (speedup 1.0000x reference)
"""Multi-head cross attention on 8 Trainium2 NeuronCores.

Sharding: core c = b*4 + g handles batch b (of 2) and head-group g (4 heads
of the 16).  Each core projects Q/K/V for its 4 heads, runs attention, and
computes a partial output projection with its 256 rows of Wo; the host sums
the 4 partials per batch (plus bo and the bv@Wo term, exact because softmax
rows sum to 1).

Dataflow is fully "transposed" so no on-device transposes are needed:
  - host passes x^T in bf16 (transposed + cast on CPU)
  - QT[dh, sq]  = Wq_g.T @ xqT       (lhsT = Wq slice, rhs = xqT)
  - KT[dh, skv] = Wk_g.T @ xkvT      (two heads packed per 128-partition tile)
  - V[skv, dh]  = xkvT.T @ Wv'_g     (lhsT = xkvT slice, rhs = Wv' which has
                                      a zero column after each head, turned
                                      into ones -> fused softmax row-sums)
  - S^T[skv, q] = KT_h.T @ QT_h      (K=64; the two heads of a pair use
                                      disjoint PE row groups and execute
                                      concurrently; both land in one 1024-wide
                                      PSUM tile so one ACT Exp covers both)
  - P^T = exp(S^T / 8)               (no max subtraction; |scores/8| < ~3)
  - O'^T = [V_h|1|...].T @ P^T       (lhsT window is 128 wide so the PE array
                                      is fully used and the HAM clock gate
                                      stays at 2.4 GHz; rows 65-127 are junk,
                                      row 64 is the softmax row-sum)
  - O^T = O'^T[0:64] * (1/rowsum)    (reciprocal on DVE, broadcast across
                                      partitions on the idle GpSimd engine,
                                      multiply on DVE -- nothing touches the
                                      PE queue or PSUM, so block boundaries
                                      don't stall the matmul pipeline)
  - out_partial[sq, 1024] = O^T_allheads.T @ Wo_g
Attention runs in 8 blocks (head-pair x query-quarter); each block's
normalize is emitted two kv-tiles into the next block so its instructions
sit behind fresh matmul work in every engine queue.
Matmuls run in bf16 (fp32 PSUM accumulation; measured rel err ~4e-3).
fp32r (1 cyc/row in the cost model) measured ~3.5 cyc/row on HW and is
throttled in exactly the shapes attention needs, so bf16 wins.
"""

import sys

sys.path.insert(0, "/opt/trn_rl_repo")

import ml_dtypes
import numpy as np

BF16NP = ml_dtypes.bfloat16

B, SQ, SKV, D, H = 2, 2048, 2048, 1024, 16
DH = D // H          # 64
N_CORES = 8
G = 4                # head groups
HPG = H // G         # heads per group = 4
GC = HPG * DH        # group width = 256

_nc_cache = None


def _build_nc():
    import concourse.mybir as mybir
    import concourse.tile as tile
    from concourse import bacc

    F32 = mybir.dt.float32
    F32R = mybir.dt.float32r
    BF16 = mybir.dt.bfloat16
    AF = mybir.ActivationFunctionType
    MUL = mybir.AluOpType.mult

    nc = bacc.Bacc("TRN2", target_bir_lowering=False, debug=False,
                   num_devices=N_CORES)

    xqT_d = nc.dram_tensor("xqT", [D, SQ], BF16, kind="ExternalInput").ap()
    xkvT_d = nc.dram_tensor("xkvT", [D, SKV], BF16, kind="ExternalInput").ap()
    wq_d = nc.dram_tensor("wq", [D, GC], BF16, kind="ExternalInput").ap()
    wk_d = nc.dram_tensor("wk", [D, GC], BF16, kind="ExternalInput").ap()
    # Wv' with a zero column after each head's 64 (slots for the ones column)
    wvp_d = nc.dram_tensor("wvp", [D, HPG * 65], BF16, kind="ExternalInput").ap()
    wo_d = nc.dram_tensor("wo", [GC, D], BF16, kind="ExternalInput").ap()
    bq_d = nc.dram_tensor("bq2", [128, 2], F32, kind="ExternalInput").ap()
    bk_d = nc.dram_tensor("bk2", [128, 2], F32, kind="ExternalInput").ap()
    ones_d = nc.dram_tensor("ones64", [1, 128], F32R, kind="ExternalInput").ap()
    out_d = nc.dram_tensor("out_p", [SQ, D], F32, kind="ExternalOutput").ap()

    ND = D // 128        # 8 d-tiles (contraction over D)
    NJ = SKV // 128      # 16 kv tiles
    VW = HPG * 65        # 260, V' row width
    scale = 1.0 / float(np.sqrt(DH))

    with tile.TileContext(nc) as tc:
        with (
            tc.tile_pool(name="persist", bufs=1) as pp,
            tc.tile_pool(name="pha", bufs=1) as pa,
            tc.tile_pool(name="phb", bufs=1) as pb,
        ):
            # ---- persistent tiles -------------------------------------
            qt_sb = pp.tile([128, 2 * SQ], BF16, tag="qt_sb")
            kt_sb = pp.tile([128, 2 * SKV], BF16, tag="kt_sb")
            vp_sb = pp.tile([128, NJ * VW + 63], BF16, tag="vp_sb")
            o_sbA = pp.tile([128, 2 * 1024], BF16, tag="o_sbA")
            o_sbB = pp.tile([128, 2 * 1024], BF16, tag="o_sbB")
            bq_sb = pp.tile([128, 2], F32, tag="bq_sb")
            bk_sb = pp.tile([128, 2], F32, tag="bk_sb")
            ones_sb = pp.tile([1, 128], F32R, tag="ones_sb")
            nc.sync.dma_start(out=bq_sb[:], in_=bq_d[:])
            nc.sync.dma_start(out=bk_sb[:], in_=bk_d[:])
            nc.sync.dma_start(out=ones_sb[:], in_=ones_d[:])

            # ---- phase A: load xkvT, weights; compute KT and V' -------
            wk_sb = pa.tile([128, ND * GC], BF16, tag="wk_sb")
            wvp_sb = pa.tile([128, ND * VW], BF16, tag="wvp_sb")
            for d in range(ND):
                nc.sync.dma_start(
                    out=wk_sb[:, d * GC:(d + 1) * GC],
                    in_=wk_d[d * 128:(d + 1) * 128, :])
            xkv = []
            for d in range(ND):
                t = pa.tile([128, SKV], BF16, tag=f"xkv{d}", name=f"xkv{d}")
                nc.gpsimd.dma_start(out=t[:], in_=xkvT_d[d * 128:(d + 1) * 128, :])
                xkv.append(t)

            with tc.tile_pool(name="psA", bufs=1, space="PSUM") as psA:
                # KT (2 pair-tiles x 4 q chunks); d-outer so each xkv DMA
                # tile is consumed as soon as it lands
                pk = {}
                for p in range(2):
                    for qc in range(4):
                        pk[p, qc] = psA.tile([128, 512], F32, tag="pk",
                                             bufs=8, name=f"pk{p}{qc}")
                for d in range(ND):
                    for p in range(2):
                        for qc in range(4):
                            nc.tensor.matmul(
                                pk[p, qc][:],
                                wk_sb[:, d * GC + p * 128:d * GC + (p + 1) * 128],
                                xkv[d][:, qc * 512:(qc + 1) * 512],
                                start=(d == 0), stop=(d == ND - 1),
                            )
                for p in range(2):
                    for qc in range(4):
                        nc.scalar.activation(
                            kt_sb[:, p * SKV + qc * 512:p * SKV + (qc + 1) * 512],
                            pk[p, qc][:], AF.Identity, bias=bk_sb[:, p:p + 1])
                # V' (16 kv tiles, accumulate over d)
                for d in range(ND):
                    nc.sync.dma_start(
                        out=wvp_sb[:, d * VW:(d + 1) * VW],
                        in_=wvp_d[d * 128:(d + 1) * 128, :])
                for j in range(NJ):
                    pv = psA.tile([128, VW], F32, tag="pk", bufs=8,
                                  name=f"pv{j}")
                    for d in range(ND):
                        nc.tensor.matmul(
                            pv[:],
                            xkv[d][:, j * 128:(j + 1) * 128],
                            wvp_sb[:, d * VW:(d + 1) * VW],
                            start=(d == 0), stop=(d == ND - 1),
                        )
                    nc.vector.tensor_copy(vp_sb[:, j * VW:(j + 1) * VW], pv[:])
                # ones columns of V' (stride-65 view hits col 64 of each head)
                oc = vp_sb[:, 64:NJ * VW:65]
                nc.scalar.activation(oc, oc, AF.Copy, scale=0.0, bias=1.0)
                # zero tail pad (scale-0 copy from finite psum keeps NaNs out)
                nc.scalar.activation(vp_sb[:, NJ * VW:NJ * VW + 63],
                                     pv[:, 0:63], AF.Copy, scale=0.0)

                # ---- phase B: stream xqT, compute QT ------------------
                wq_sb = pb.tile([128, ND * GC], BF16, tag="wq_sb")
                for d in range(ND):
                    nc.sync.dma_start(
                        out=wq_sb[:, d * GC:(d + 1) * GC],
                        in_=wq_d[d * 128:(d + 1) * 128, :])
                xq_tiles = []
                for d in range(ND):
                    xq_t = pb.tile([128, SQ], BF16, tag="xq", bufs=3,
                                   name=f"xq{d}")
                    nc.gpsimd.dma_start(out=xq_t[:],
                                        in_=xqT_d[d * 128:(d + 1) * 128, :])
                    xq_tiles.append(xq_t)
                pq = {}
                for p in range(2):
                    for qc in range(4):
                        pq[p, qc] = psA.tile([128, 512], F32, tag="pk", bufs=8,
                                             name=f"pq{p}{qc}")
                for d in range(ND):
                    xq_t = xq_tiles[d]
                    for p in range(2):
                        for qc in range(4):
                            nc.tensor.matmul(
                                pq[p, qc][:],
                                wq_sb[:, d * GC + p * 128:d * GC + (p + 1) * 128],
                                xq_t[:, qc * 512:(qc + 1) * 512],
                                start=(d == 0), stop=(d == ND - 1),
                            )
                for p in range(2):
                    for qc in range(4):
                        blk = slice(p * SQ + qc * 512, p * SQ + (qc + 1) * 512)
                        nc.scalar.activation(
                            qt_sb[:, blk], pq[p, qc][:],
                            AF.Identity, bias=bq_sb[:, p:p + 1])

            # ---- attention -------------------------------------------
            with (
                tc.tile_pool(name="attn", bufs=1) as at,
                tc.tile_pool(name="psC", bufs=1, space="PSUM") as psC,
                tc.tile_pool(name="oproj", bufs=1) as op_pool,
                tc.tile_pool(name="psD", bufs=1, space="PSUM") as psD,
            ):
                wo_sb = op_pool.tile([128, 2 * D], BF16, tag="wo_sb")
                nc.sync.dma_start(
                    out=wo_sb[:].rearrange("p (t n) -> p t n", t=2),
                    in_=wo_d.rearrange("(t p) n -> p t n", p=128),
                )

                def emit_outproj(lo, hi):
                    for s in range(lo, hi):
                        for n2 in range(2):
                            po = psD.tile([128, 512], F32, tag="po", bufs=2,
                                          name=f"po{s}{n2}")
                            o_half = o_sbA if s < 8 else o_sbB
                            s8 = s % 8
                            for tt in range(2):
                                nc.tensor.matmul(
                                    po[:],
                                    o_half[:, tt * 1024 + s8 * 128:
                                           tt * 1024 + (s8 + 1) * 128],
                                    wo_sb[:, tt * D + n2 * 512:
                                          tt * D + n2 * 512 + 512],
                                    start=(tt == 0), stop=(tt == 1),
                                )
                            ob = op_pool.tile([128, 512], F32, tag="ob",
                                              bufs=3, name=f"ob{s}{n2}")
                            nc.vector.tensor_copy(ob[:], po[:])
                            nc.sync.dma_start(
                                out=out_d[s * 128:(s + 1) * 128,
                                          n2 * 512:(n2 + 1) * 512],
                                in_=ob[:])

                pending_norm = []

                def flush_norm():
                    while pending_norm:
                        pending_norm.pop(0)()

                for t in range(2):          # head pair
                    for qq in range(4):     # q quarter (512)
                        o_ps = {}
                        for hp in range(2):
                            o_ps[hp] = psC.tile(
                                [128, 512], F32, tag="o_ps", bufs=2,
                                name=f"o_ps{t}{qq}{hp}")
                        for j in range(NJ):
                            st = psC.tile([128, 1024], F32, tag="st2", bufs=2,
                                          name=f"st{t}{qq}{j}")
                            # K=64 scores; the two heads use disjoint row
                            # groups (partitions 0-63 / 64-127) and execute
                            # concurrently on the PE
                            for hp in range(2):
                                nc.tensor.matmul(
                                    st[:, hp * 512:(hp + 1) * 512],
                                    kt_sb[hp * 64:(hp + 1) * 64,
                                          t * SKV + j * 128:
                                          t * SKV + (j + 1) * 128],
                                    qt_sb[hp * 64:(hp + 1) * 64,
                                          t * SQ + qq * 512:
                                          t * SQ + (qq + 1) * 512],
                                    start=True, stop=True,
                                )
                            p_t = at.tile([128, 1024], BF16, tag="pt",
                                          bufs=6, name=f"pt{t}{qq}{j}")
                            nc.scalar.activation(p_t[:], st[:],
                                                 AF.Exp, scale=scale)
                            for hp in range(2):
                                h = 2 * t + hp
                                nc.tensor.matmul(
                                    o_ps[hp][:],
                                    vp_sb[:, j * VW + h * 65:
                                          j * VW + h * 65 + 128],
                                    p_t[:, hp * 512:(hp + 1) * 512],
                                    start=(j == 0), stop=(j == NJ - 1),
                                )
                            if j == 1 and t == 1 and qq == 3:
                                # o_sbA's last normalize (t1,qq1) is already
                                # emitted; its outproj half can gap-fill the
                                # PE during the final attention blocks
                                flush_norm()
                                emit_outproj(0, 8)
                            elif j == 1:
                                # emit the previous block's normalize now --
                                # its bc matmuls land behind this block's
                                # first scores in the PE stream, so the PE
                                # never head-of-line blocks on the slow
                                # reciprocal chain
                                flush_norm()
                        # stage rowsums out of PSUM quickly, then queue the
                        # rest of the normalize for later emission
                        for hp in range(2):
                            ot = at.tile([64, 512], F32, tag="ot",
                                         bufs=4, name=f"ot{t}{qq}{hp}")
                            nc.vector.tensor_copy(ot[:], o_ps[hp][0:64, :])
                            rs = at.tile([1, 512], F32, tag="rs", bufs=4,
                                         name=f"rs{t}{qq}{hp}")
                            nc.vector.tensor_copy(rs[:], o_ps[hp][64:65, :])

                            def norm(t=t, qq=qq, hp=hp, ot=ot, rs=rs):
                                rcp = at.tile([1, 512], F32, tag="rcp",
                                              bufs=4, name=f"rcp{t}{qq}{hp}")
                                nc.vector.reciprocal(rcp[:], rs[:])
                                bcs = at.tile([64, 512], F32, tag="bcs",
                                              bufs=4, name=f"bcs{t}{qq}{hp}")
                                nc.gpsimd.partition_broadcast(
                                    bcs[:], rcp[:], channels=64)
                                o_half = o_sbA if qq < 2 else o_sbB
                                col = t * 1024 + (qq % 2) * 512
                                nc.vector.tensor_tensor(
                                    out=o_half[hp * 64:(hp + 1) * 64,
                                               col:col + 512],
                                    in0=ot[:], in1=bcs[:],
                                    op=MUL)

                            pending_norm.append(norm)
                flush_norm()

                # ---- output projection (second half; first half was
                # emitted inside the attention loop) ------------------------
                emit_outproj(8, 16)

    nc.compile()
    return nc


def build_in_maps(inputs):
    query_input = np.asarray(inputs["query_input"], dtype=np.float32)
    kv_input = np.asarray(inputs["kv_input"], dtype=np.float32)
    Wq = np.asarray(inputs["Wq"], dtype=np.float32)
    bq = np.asarray(inputs["bq"], dtype=np.float32)
    Wkv = np.asarray(inputs["Wkv"], dtype=np.float32)
    bkv = np.asarray(inputs["bkv"], dtype=np.float32)
    Wo = np.asarray(inputs["Wo"], dtype=np.float32)

    Wk = Wkv[:, :D]
    Wv = Wkv[:, D:]
    bk = bkv[:D]
    ones64 = np.ones((1, 128), np.float32)

    xT = [np.ascontiguousarray(query_input[b].T).astype(BF16NP) for b in range(B)]
    kvT = [np.ascontiguousarray(kv_input[b].T).astype(BF16NP) for b in range(B)]

    in_maps = []
    for c in range(N_CORES):
        b, g = divmod(c, G)
        c0 = g * GC
        wvp = np.zeros((D, HPG * 65), np.float32)
        for h in range(HPG):
                wvp[:, h * 65:h * 65 + 64] = Wv[:, c0 + h * DH:c0 + (h + 1) * DH]
        bq2 = bq[c0:c0 + GC].reshape(2, 128).T.copy()
        bk2 = bk[c0:c0 + GC].reshape(2, 128).T.copy()
        in_maps.append({
                "xqT": xT[b],
                "xkvT": kvT[b],
                "wq": np.ascontiguousarray(Wq[:, c0:c0 + GC]).astype(BF16NP),
                "wk": np.ascontiguousarray(Wk[:, c0:c0 + GC]).astype(BF16NP),
                "wvp": wvp.astype(BF16NP),
                "wo": np.ascontiguousarray(Wo[c0:c0 + GC, :]).astype(BF16NP),
                "bq2": np.ascontiguousarray(bq2),
                "bk2": np.ascontiguousarray(bk2),
                "ones64": ones64,
        })
    return in_maps


def kernel(query_input, kv_input, Wq, bq, Wkv, bkv, Wo, bo):
    global _nc_cache
    from concourse import bass_utils

    if _nc_cache is None:
        _nc_cache = _build_nc()
    nc = _nc_cache

    Wkv = np.asarray(Wkv, dtype=np.float32)
    Wo = np.asarray(Wo, dtype=np.float32)
    bo = np.asarray(bo, dtype=np.float32)
    bv = np.asarray(bkv, np.float32)[D:]

    in_maps = build_in_maps(dict(
        query_input=query_input, kv_input=kv_input, Wq=Wq, bq=bq,
        Wkv=Wkv, bkv=bkv, Wo=Wo))

    res = bass_utils.run_bass_kernel_spmd(nc, in_maps,
                                          core_ids=list(range(N_CORES)))

    # gather: sum the 4 head-group partials per batch; add biases the device
    # left out (bo, and bv which passes through Wo since softmax rows sum to 1)
    tail = bv @ Wo + bo
    out = np.empty((B, SQ, D), np.float32)
    for b in range(B):
        acc = res.results[b * G + 0]["out_p"].astype(np.float32).copy()
        for g in range(1, G):
                acc += res.results[b * G + g]["out_p"]
        out[b] = acc + tail[None, :]
    return out



# revision 7
# speedup vs baseline: 1.0898x; 1.0898x over previous
"""Multi-head cross attention on 8 Trainium2 NeuronCores.

Sharding: core c = b*4 + g handles batch b (of 2) and head-group g (4 heads
of the 16).  Each core projects Q/K/V for its 4 heads, runs attention, and
computes a partial output projection with its 256 rows of Wo; the host sums
the 4 partials per batch (plus bo and the bv@Wo term, exact because softmax
rows sum to 1).

Schedule (v2): the ACT engine's exp over 16.8M score elements (~147us at
1 elem/cyc/lane) is the per-core bottleneck, so everything else hides
under it:
  - weights load in single-descriptor DMAs; xkvT/xqT stream as 8 per-d
    tiles so K/Q projections start as tiles land (PE busy from ~5us)
  - ACT runs exp ONLY (table preloaded via a dummy exp at t=0); the
    KT/QT bias-adds run on DVE, the V'-ones fixup on GpSimd memset
  - attention blocks iterate qq-outer/t-inner; the V' projection is
    woven per-j into the first block and the output projection for each
    finished qq group is woven into the next group's first block, so the
    PE gap-fills while ACT streams exps back-to-back
  - softmax normalize: one [65,512] PSUM copy, reciprocal_approx_fast
    (5x the old reciprocal), GpSimd partition-broadcast, DVE multiply
Dataflow is fully "transposed" so no on-device transposes are needed
(see build_in_maps): QT/KT = W.T @ xT, V' = xkvT.T @ Wv' with a ones
column per head that makes the AV matmul also emit softmax row-sums.
Matmuls run in bf16 (fp32 PSUM accumulation; measured rel err ~4e-3).
"""

import sys

sys.path.insert(0, "/opt/trn_rl_repo")

import ml_dtypes
import numpy as np

BF16NP = ml_dtypes.bfloat16

B, SQ, SKV, D, H = 2, 2048, 2048, 1024, 16
DH = D // H          # 64
N_CORES = 8
G = 4                # head groups
HPG = H // G         # heads per group = 4
GC = HPG * DH        # group width = 256

_nc_cache = None


def _build_nc():
    import concourse.mybir as mybir
    import concourse.tile as tile
    from concourse import bacc

    F32 = mybir.dt.float32
    BF16 = mybir.dt.bfloat16
    AF = mybir.ActivationFunctionType
    MUL = mybir.AluOpType.mult

    nc = bacc.Bacc("TRN2", target_bir_lowering=False, debug=False,
                   num_devices=N_CORES)

    xqT_d = nc.dram_tensor("xqT", [D, SQ], BF16, kind="ExternalInput").ap()
    xkvT_d = nc.dram_tensor("xkvT", [D, SKV], BF16, kind="ExternalInput").ap()
    wq_d = nc.dram_tensor("wq", [D, GC], BF16, kind="ExternalInput").ap()
    wk_d = nc.dram_tensor("wk", [D, GC], BF16, kind="ExternalInput").ap()
    # Wv' with a zero column after each head's 64 (slots for the ones column)
    wvp_d = nc.dram_tensor("wvp", [D, HPG * 65], BF16, kind="ExternalInput").ap()
    wo_d = nc.dram_tensor("wo", [GC, D], BF16, kind="ExternalInput").ap()
    bq_d = nc.dram_tensor("bq2", [128, 2], F32, kind="ExternalInput").ap()
    bk_d = nc.dram_tensor("bk2", [128, 2], F32, kind="ExternalInput").ap()
    # legacy input, unused on-device but still part of the host contract
    nc.dram_tensor("ones64", [1, 128], mybir.dt.float32r, kind="ExternalInput")
    out_d = nc.dram_tensor("out_p", [SQ, D], F32, kind="ExternalOutput").ap()
    _DBG = bool(globals().get("_DEBUG_DUMPS"))
    if _DBG:
        dbg_kt = nc.dram_tensor("dbg_kt", [128, 2 * SKV], mybir.dt.bfloat16,
                                kind="ExternalOutput").ap()
        dbg_qt = nc.dram_tensor("dbg_qt", [128, 2 * SQ], mybir.dt.bfloat16,
                                kind="ExternalOutput").ap()
        dbg_vp = nc.dram_tensor("dbg_vp", [128, (SKV // 128) * HPG * 65 + 63],
                                mybir.dt.bfloat16, kind="ExternalOutput").ap()
        dbg_oA = nc.dram_tensor("dbg_oA", [128, 2 * 1024], mybir.dt.bfloat16,
                                kind="ExternalOutput").ap()
        dbg_oB = nc.dram_tensor("dbg_oB", [128, 2 * 1024], mybir.dt.bfloat16,
                                kind="ExternalOutput").ap()

    ND = D // 128        # 8 d-tiles (contraction over D)
    NJ = SKV // 128      # 16 kv tiles
    VW = HPG * 65        # 260, V' row width
    scale = 1.0 / float(np.sqrt(DH))

    with tile.TileContext(nc) as tc:
        with (
            tc.tile_pool(name="persist", bufs=1) as pp,
            tc.tile_pool(name="attn", bufs=1) as at,
        ):
            # ---- persistent tiles -------------------------------------
            qt_sb = pp.tile([128, 2 * SQ], BF16, tag="qt_sb")
            kt_sb = pp.tile([128, 2 * SKV], BF16, tag="kt_sb")
            vp_sb = pp.tile([128, NJ * VW + 63], BF16, tag="vp_sb")
            o_sbA = pp.tile([128, 2 * 1024], BF16, tag="o_sbA")
            o_sbB = pp.tile([128, 2 * 1024], BF16, tag="o_sbB")
            bq_sb = pp.tile([128, 2], F32, tag="bq_sb")
            bk_sb = pp.tile([128, 2], F32, tag="bk_sb")
            wk_sb = pp.tile([128, ND * GC], BF16, tag="wk_sb")
            wq_sb = pp.tile([128, ND * GC], BF16, tag="wq_sb")
            wvp_sb = pp.tile([128, ND * VW], BF16, tag="wvp_sb")
            wo_sb = pp.tile([128, 2 * D], BF16, tag="wo_sb")
            warm = pp.tile([1, 32], F32, tag="warm")

            # preload the exp spline tables while DMAs stream (ACT does
            # nothing else until the first score exp)
            nc.gpsimd.memset(warm[:], 0.0)
            nc.scalar.activation(warm[:], warm[:], AF.Exp)
            # zero the 63-col tail pad of V' (AV lhsT windows over-read it)
            nc.gpsimd.memset(vp_sb[:, NJ * VW:NJ * VW + 63], 0.0)

            # ---- DMA issue (order = priority) -------------------------
            # sync queue: weights (single descriptors each)
            nc.sync.dma_start(
                out=wk_sb[:].rearrange("p (t n) -> p t n", t=ND),
                in_=wk_d.rearrange("(t p) n -> p t n", p=128))
            nc.sync.dma_start(
                out=wq_sb[:].rearrange("p (t n) -> p t n", t=ND),
                in_=wq_d.rearrange("(t p) n -> p t n", p=128))
            nc.sync.dma_start(out=bq_sb[:], in_=bq_d[:])
            nc.sync.dma_start(out=bk_sb[:], in_=bk_d[:])
            nc.sync.dma_start(
                out=wvp_sb[:].rearrange("p (t n) -> p t n", t=ND),
                in_=wvp_d.rearrange("(t p) n -> p t n", p=128))
            nc.sync.dma_start(
                out=wo_sb[:].rearrange("p (t n) -> p t n", t=2),
                in_=wo_d.rearrange("(t p) n -> p t n", p=128))
            # gpsimd queue: activations, per-d tiles for arrival-chained use
            xkv = []
            for d in range(ND):
                t = pp.tile([128, SKV], BF16, tag=f"xkv{d}", name=f"xkv{d}")
                nc.gpsimd.dma_start(out=t[:], in_=xkvT_d[d * 128:(d + 1) * 128, :])
                xkv.append(t)
            xq = []
            for d in range(ND):
                t = pp.tile([128, SQ], BF16, tag=f"xq{d}", name=f"xq{d}")
                nc.gpsimd.dma_start(out=t[:], in_=xqT_d[d * 128:(d + 1) * 128, :])
                xq.append(t)

            # ---- K and Q projections (PE from ~5us) -------------------
            with tc.tile_pool(name="psA", bufs=1, space="PSUM") as psA:
                pk = {}
                for p in range(2):
                    for qc in range(4):
                        pk[p, qc] = psA.tile([128, 512], F32, tag="pk",
                                             bufs=8, name=f"pk{p}{qc}")
                for d in range(ND):
                    for p in range(2):
                        for qc in range(4):
                            nc.tensor.matmul(
                                pk[p, qc][:],
                                wk_sb[:, d * GC + p * 128:d * GC + (p + 1) * 128],
                                xkv[d][:, qc * 512:(qc + 1) * 512],
                                start=(d == 0), stop=(d == ND - 1),
                            )
                for p in range(2):
                    for qc in range(4):
                        nc.vector.tensor_scalar_add(
                            kt_sb[:, p * SKV + qc * 512:p * SKV + (qc + 1) * 512],
                            pk[p, qc][:], bk_sb[:, p:p + 1])
                pq = {}
                for p in range(2):
                    for qc in range(4):
                        pq[p, qc] = psA.tile([128, 512], F32, tag="pk", bufs=8,
                                             name=f"pq{p}{qc}")
                for d in range(ND):
                    for p in range(2):
                        for qc in range(4):
                            nc.tensor.matmul(
                                pq[p, qc][:],
                                wq_sb[:, d * GC + p * 128:d * GC + (p + 1) * 128],
                                xq[d][:, qc * 512:(qc + 1) * 512],
                                start=(d == 0), stop=(d == ND - 1),
                            )
                for p in range(2):
                    for qc in range(4):
                        nc.vector.tensor_scalar_add(
                            qt_sb[:, p * SQ + qc * 512:p * SQ + (qc + 1) * 512],
                            pq[p, qc][:], bq_sb[:, p:p + 1])

            # ---- attention (ACT-bound; PE gap-fills V'/out projections)
            with tc.tile_pool(name="psC", bufs=1, space="PSUM") as psC:
                pending_norm = []

                def flush_norm():
                    while pending_norm:
                        pending_norm.pop(0)()

                def emit_vproj(j):
                    # V' tile j: accumulate over d, copy to vp_sb, set the
                    # ones column of each head (stride-65 view)
                    pv = psC.tile([128, 512], F32, tag="aux", bufs=2,
                                  name=f"pv{j}")
                    for d in range(ND):
                        nc.tensor.matmul(
                            pv[:, 0:VW],
                            xkv[d][:, j * 128:(j + 1) * 128],
                            wvp_sb[:, d * VW:(d + 1) * VW],
                            start=(d == 0), stop=(d == ND - 1),
                        )
                    nc.vector.tensor_copy(vp_sb[:, j * VW:(j + 1) * VW],
                                          pv[:, 0:VW])
                    nc.gpsimd.memset(
                        vp_sb[:, j * VW + 64:(j + 1) * VW:65], 1.0)

                def emit_outproj_tile(s, n2):
                    po = psC.tile([128, 512], F32, tag="aux", bufs=2,
                                  name=f"po{s}{n2}")
                    o_half = o_sbA if s < 8 else o_sbB
                    s8 = s % 8
                    for tt in range(2):
                        nc.tensor.matmul(
                            po[:],
                            o_half[:, tt * 1024 + s8 * 128:
                                   tt * 1024 + (s8 + 1) * 128],
                            wo_sb[:, tt * D + n2 * 512:
                                  tt * D + n2 * 512 + 512],
                            start=(tt == 0), stop=(tt == 1),
                        )
                    ob = at.tile([128, 512], F32, tag="ob",
                                 bufs=3, name=f"ob{s}{n2}")
                    nc.vector.tensor_copy(ob[:], po[:])
                    nc.sync.dma_start(
                        out=out_d[s * 128:(s + 1) * 128,
                                  n2 * 512:(n2 + 1) * 512],
                        in_=ob[:])

                for qq in range(4):         # q quarter (512)
                    for t in range(2):      # head pair
                        # weave list of thunks for this block's j slots
                        weave = {}
                        if qq == 0 and t == 0:
                            for j in range(NJ):
                                weave[j] = (lambda j=j: emit_vproj(j))
                        elif t == 0:
                            # output projection of the previous qq group
                            # (both head pairs normalized by now)
                            for i, s in enumerate(range((qq - 1) * 4, qq * 4)):
                                for n2 in range(2):
                                    weave[2 + i * 2 + n2] = (
                                        lambda s=s, n2=n2:
                                        emit_outproj_tile(s, n2))
                        o_ps = {}
                        for hp in range(2):
                            o_ps[hp] = psC.tile(
                                [128, 512], F32, tag="o_ps", bufs=2,
                                name=f"o_ps{t}{qq}{hp}")
                        for j in range(NJ):
                            if j in weave:
                                weave[j]()
                            st = psC.tile([128, 1024], F32, tag="st2", bufs=2,
                                          name=f"st{t}{qq}{j}")
                            # K=64 scores; the two heads use disjoint PE row
                            # groups (partitions 0-63 / 64-127) and execute
                            # concurrently
                            for hp in range(2):
                                nc.tensor.matmul(
                                    st[:, hp * 512:(hp + 1) * 512],
                                    kt_sb[hp * 64:(hp + 1) * 64,
                                          t * SKV + j * 128:
                                          t * SKV + (j + 1) * 128],
                                    qt_sb[hp * 64:(hp + 1) * 64,
                                          t * SQ + qq * 512:
                                          t * SQ + (qq + 1) * 512],
                                    start=True, stop=True,
                                )
                            p_t = at.tile([128, 1024], BF16, tag="pt",
                                          bufs=6, name=f"pt{t}{qq}{j}")
                            nc.scalar.activation(p_t[:], st[:],
                                                 AF.Exp, scale=scale)
                            for hp in range(2):
                                h = 2 * t + hp
                                nc.tensor.matmul(
                                    o_ps[hp][:],
                                    vp_sb[:, j * VW + h * 65:
                                          j * VW + h * 65 + 128],
                                    p_t[:, hp * 512:(hp + 1) * 512],
                                    start=(j == 0), stop=(j == NJ - 1),
                                )
                            if j == 1:
                                # previous block's normalize lands behind
                                # this block's fresh matmul work
                                flush_norm()
                        # stage O'+rowsum out of PSUM, defer the normalize
                        for hp in range(2):
                            ot = at.tile([64, 512], F32, tag="ot",
                                         bufs=4, name=f"ot{t}{qq}{hp}")
                            nc.vector.tensor_copy(ot[:], o_ps[hp][0:64, :])
                            rs = at.tile([1, 512], F32, tag="rs", bufs=4,
                                         name=f"rs{t}{qq}{hp}")
                            nc.vector.tensor_copy(rs[:], o_ps[hp][64:65, :])

                            def norm(t=t, qq=qq, hp=hp, ot=ot, rs=rs):
                                rcp = at.tile([1, 512], F32, tag="rcp",
                                              bufs=4, name=f"rcp{t}{qq}{hp}")
                                nc.vector.reciprocal_approx_fast(
                                    out=rcp[:], in_=rs[:])
                                bcs = at.tile([64, 512], F32, tag="bcs",
                                              bufs=4, name=f"bcs{t}{qq}{hp}")
                                nc.gpsimd.partition_broadcast(
                                    bcs[:], rcp[:], channels=64)
                                o_half = o_sbA if qq < 2 else o_sbB
                                col = t * 1024 + (qq % 2) * 512
                                nc.vector.tensor_tensor(
                                    out=o_half[hp * 64:(hp + 1) * 64,
                                               col:col + 512],
                                    in0=ot[:], in1=bcs[:],
                                    op=MUL)

                            pending_norm.append(norm)
                flush_norm()

                # ---- last qq group's output projection --------------------
                for s in range(12, 16):
                    for n2 in range(2):
                        emit_outproj_tile(s, n2)

                if _DBG:
                    nc.sync.dma_start(out=dbg_kt[:], in_=kt_sb[:])
                    nc.sync.dma_start(out=dbg_qt[:], in_=qt_sb[:])
                    nc.sync.dma_start(out=dbg_vp[:], in_=vp_sb[:])
                    nc.sync.dma_start(out=dbg_oA[:], in_=o_sbA[:])
                    nc.sync.dma_start(out=dbg_oB[:], in_=o_sbB[:])

    nc.compile()
    return nc


def build_in_maps(inputs):
    query_input = np.asarray(inputs["query_input"], dtype=np.float32)
    kv_input = np.asarray(inputs["kv_input"], dtype=np.float32)
    Wq = np.asarray(inputs["Wq"], dtype=np.float32)
    bq = np.asarray(inputs["bq"], dtype=np.float32)
    Wkv = np.asarray(inputs["Wkv"], dtype=np.float32)
    bkv = np.asarray(inputs["bkv"], dtype=np.float32)
    Wo = np.asarray(inputs["Wo"], dtype=np.float32)

    Wk = Wkv[:, :D]
    Wv = Wkv[:, D:]
    bk = bkv[:D]
    ones64 = np.ones((1, 128), np.float32)

    xT = [np.ascontiguousarray(query_input[b].T).astype(BF16NP) for b in range(B)]
    kvT = [np.ascontiguousarray(kv_input[b].T).astype(BF16NP) for b in range(B)]

    in_maps = []
    for c in range(N_CORES):
        b, g = divmod(c, G)
        c0 = g * GC
        wvp = np.zeros((D, HPG * 65), np.float32)
        for h in range(HPG):
                wvp[:, h * 65:h * 65 + 64] = Wv[:, c0 + h * DH:c0 + (h + 1) * DH]
        bq2 = bq[c0:c0 + GC].reshape(2, 128).T.copy()
        bk2 = bk[c0:c0 + GC].reshape(2, 128).T.copy()
        in_maps.append({
                "xqT": xT[b],
                "xkvT": kvT[b],
                "wq": np.ascontiguousarray(Wq[:, c0:c0 + GC]).astype(BF16NP),
                "wk": np.ascontiguousarray(Wk[:, c0:c0 + GC]).astype(BF16NP),
                "wvp": wvp.astype(BF16NP),
                "wo": np.ascontiguousarray(Wo[c0:c0 + GC, :]).astype(BF16NP),
                "bq2": np.ascontiguousarray(bq2),
                "bk2": np.ascontiguousarray(bk2),
                "ones64": ones64,
        })
    return in_maps


def kernel(query_input, kv_input, Wq, bq, Wkv, bkv, Wo, bo):
    global _nc_cache
    from concourse import bass_utils

    if _nc_cache is None:
        _nc_cache = _build_nc()
    nc = _nc_cache

    Wkv = np.asarray(Wkv, dtype=np.float32)
    Wo = np.asarray(Wo, dtype=np.float32)
    bo = np.asarray(bo, dtype=np.float32)
    bv = np.asarray(bkv, np.float32)[D:]

    in_maps = build_in_maps(dict(
        query_input=query_input, kv_input=kv_input, Wq=Wq, bq=bq,
        Wkv=Wkv, bkv=bkv, Wo=Wo))

    res = bass_utils.run_bass_kernel_spmd(nc, in_maps,
                                          core_ids=list(range(N_CORES)))

    # gather: sum the 4 head-group partials per batch; add biases the device
    # left out (bo, and bv which passes through Wo since softmax rows sum to 1)
    tail = bv @ Wo + bo
    out = np.empty((B, SQ, D), np.float32)
    for b in range(B):
        acc = res.results[b * G + 0]["out_p"].astype(np.float32).copy()
        for g in range(1, G):
                acc += res.results[b * G + g]["out_p"]
        out[b] = acc + tail[None, :]
    return out


# revision 9
# speedup vs baseline: 1.1374x; 1.0437x over previous
"""Multi-head cross attention on 8 Trainium2 NeuronCores.

Sharding: core c = b*4 + g handles batch b (of 2) and head-group g (4 heads
of the 16).  Each core projects Q/K/V for its 4 heads, runs attention, and
computes a partial output projection with its 256 rows of Wo; the host sums
the 4 partials per batch (plus bo and the bv@Wo term, exact because softmax
rows sum to 1).

Schedule (v3): the ACT engine's exp over 16.8M score elements (~147us at
1 elem/cyc/lane) is the per-core bottleneck; the whole schedule exists to
start that exp stream early and never let it gap:
  - inputs stream as paired-d tiles (xkv on the fast sync DMA queue, xq on
    gpsimd); K projection runs 2 rounds over 4 PSUM banks as tiles land,
    then only the qq0 quarter of the Q projection -> first exp at ~24us
  - attention uses a one-block software pipeline: block k emits scores+exps
    for block k and the AV matmuls of block k-1 (p_t tiles carry over), so
    exp never waits on the AV/normalize chain at block boundaries
  - the PE's per-j slack under ACT (~0.5us) is packed with fillers:
    V' projection in block 0, Q projection qc1-3 in blocks 1-2, the output
    projection of finished qq groups in blocks 3/5, block 7's own AVs in
    block 7; outproj(qq2)+(qq3) drain at the end
  - ACT does exp ONLY (table preloaded via a dummy exp at t=0); KT/QT bias
    adds run on DVE, V'-ones fixups on GpSimd memset, softmax normalize is
    reciprocal_approx_fast + partition-broadcast + DVE multiply
PSUM: st 2x[128,1024] (4 banks) + o_ps pair (2) + aux (2) shared serially
by {pv, woven pq, po, block-7 o_ps pair}.
Dataflow is fully transposed (see build_in_maps): QT/KT = W.T @ xT,
V' = xkvT.T @ Wv' with a ones column per head so the AV matmul also emits
softmax row-sums.  Matmuls in bf16, fp32 PSUM accumulation.
"""

import sys

sys.path.insert(0, "/opt/trn_rl_repo")

import ml_dtypes
import numpy as np

BF16NP = ml_dtypes.bfloat16

B, SQ, SKV, D, H = 2, 2048, 2048, 1024, 16
DH = D // H          # 64
N_CORES = 8
G = 4                # head groups
HPG = H // G         # heads per group = 4
GC = HPG * DH        # group width = 256

_nc_cache = None


def _build_nc():
    import concourse.mybir as mybir
    import concourse.tile as tile
    from concourse import bacc

    F32 = mybir.dt.float32
    BF16 = mybir.dt.bfloat16
    AF = mybir.ActivationFunctionType
    MUL = mybir.AluOpType.mult

    nc = bacc.Bacc("TRN2", target_bir_lowering=False, debug=False,
                   num_devices=N_CORES)

    xqT_d = nc.dram_tensor("xqT", [D, SQ], BF16, kind="ExternalInput").ap()
    xkvT_d = nc.dram_tensor("xkvT", [D, SKV], BF16, kind="ExternalInput").ap()
    wq_d = nc.dram_tensor("wq", [D, GC], BF16, kind="ExternalInput").ap()
    wk_d = nc.dram_tensor("wk", [D, GC], BF16, kind="ExternalInput").ap()
    # Wv' with a zero column after each head's 64 (slots for the ones column)
    wvp_d = nc.dram_tensor("wvp", [D, HPG * 65], BF16, kind="ExternalInput").ap()
    wo_d = nc.dram_tensor("wo", [GC, D], BF16, kind="ExternalInput").ap()
    bq_d = nc.dram_tensor("bq2", [128, 2], F32, kind="ExternalInput").ap()
    bk_d = nc.dram_tensor("bk2", [128, 2], F32, kind="ExternalInput").ap()
    # legacy input, unused on-device but still part of the host contract
    nc.dram_tensor("ones64", [1, 128], mybir.dt.float32r, kind="ExternalInput")
    out_d = nc.dram_tensor("out_p", [SQ, D], F32, kind="ExternalOutput").ap()
    _DBG = bool(globals().get("_DEBUG_DUMPS"))
    if _DBG:
        dbg_kt = nc.dram_tensor("dbg_kt", [128, 2 * SKV], BF16,
                                kind="ExternalOutput").ap()
        dbg_qt = nc.dram_tensor("dbg_qt", [128, 2 * SQ], BF16,
                                kind="ExternalOutput").ap()
        dbg_vp = nc.dram_tensor("dbg_vp", [128, (SKV // 128) * HPG * 65 + 63],
                                BF16, kind="ExternalOutput").ap()
        dbg_oA = nc.dram_tensor("dbg_oA", [128, 2 * 1024], BF16,
                                kind="ExternalOutput").ap()
        dbg_oB = nc.dram_tensor("dbg_oB", [128, 2 * 1024], BF16,
                                kind="ExternalOutput").ap()

    ND = D // 128        # 8 d-tiles (contraction over D)
    NP = ND // 2         # 4 paired-d input tiles
    NJ = SKV // 128      # 16 kv tiles
    VW = HPG * 65        # 260, V' row width
    scale = 1.0 / float(np.sqrt(DH))

    with tile.TileContext(nc) as tc:
        with (
            tc.tile_pool(name="persist", bufs=1) as pp,
            tc.tile_pool(name="attn", bufs=1) as at,
        ):
            # ---- persistent tiles -------------------------------------
            qt_sb = pp.tile([128, 2 * SQ], BF16, tag="qt_sb")
            kt_sb = pp.tile([128, 2 * SKV], BF16, tag="kt_sb")
            vp_sb = pp.tile([128, NJ * VW + 63], BF16, tag="vp_sb")
            o_sbA = pp.tile([128, 2 * 1024], BF16, tag="o_sbA")
            o_sbB = pp.tile([128, 2 * 1024], BF16, tag="o_sbB")
            bq_sb = pp.tile([128, 2], F32, tag="bq_sb")
            bk_sb = pp.tile([128, 2], F32, tag="bk_sb")
            wk_sb = pp.tile([128, ND * GC], BF16, tag="wk_sb")
            wq_sb = pp.tile([128, ND * GC], BF16, tag="wq_sb")
            wvp_sb = pp.tile([128, ND * VW], BF16, tag="wvp_sb")
            wo_sb = pp.tile([128, 2 * D], BF16, tag="wo_sb")
            warm = pp.tile([1, 32], F32, tag="warm")

            # preload the exp spline tables while DMAs stream
            nc.gpsimd.memset(warm[:], 0.0)
            nc.scalar.activation(warm[:], warm[:], AF.Exp)
            # zero the 63-col tail pad of V' (AV lhsT windows over-read it)
            nc.gpsimd.memset(vp_sb[:, NJ * VW:NJ * VW + 63], 0.0)

            # ---- DMA issue (order = priority) -------------------------
            # sync queue: wk first, then the xkv stream (paired-d tiles),
            # then remaining weights
            nc.sync.dma_start(
                out=wk_sb[:].rearrange("p (t n) -> p t n", t=ND),
                in_=wk_d.rearrange("(t p) n -> p t n", p=128))
            xkv = []
            for dp in range(NP):
                t = pp.tile([128, 2 * SKV], BF16, tag=f"xkv{dp}",
                            name=f"xkv{dp}")
                nc.sync.dma_start(
                    out=t[:].rearrange("p (t n) -> p t n", t=2),
                    in_=xkvT_d[dp * 256:(dp + 1) * 256, :].rearrange(
                        "(t p) n -> p t n", p=128))
                xkv.append(t)
            nc.sync.dma_start(
                out=wq_sb[:].rearrange("p (t n) -> p t n", t=ND),
                in_=wq_d.rearrange("(t p) n -> p t n", p=128))
            nc.sync.dma_start(out=bq_sb[:], in_=bq_d[:])
            nc.sync.dma_start(out=bk_sb[:], in_=bk_d[:])
            nc.sync.dma_start(
                out=wvp_sb[:].rearrange("p (t n) -> p t n", t=ND),
                in_=wvp_d.rearrange("(t p) n -> p t n", p=128))
            nc.sync.dma_start(
                out=wo_sb[:].rearrange("p (t n) -> p t n", t=2),
                in_=wo_d.rearrange("(t p) n -> p t n", p=128))
            # gpsimd queue: xq stream (paired-d tiles)
            xq = []
            for dp in range(NP):
                t = pp.tile([128, 2 * SQ], BF16, tag=f"xq{dp}", name=f"xq{dp}")
                nc.gpsimd.dma_start(
                    out=t[:].rearrange("p (t n) -> p t n", t=2),
                    in_=xqT_d[dp * 256:(dp + 1) * 256, :].rearrange(
                        "(t p) n -> p t n", p=128))
                xq.append(t)

            def xkv_ap(d, lo, hi):
                return xkv[d // 2][:, (d % 2) * SKV + lo:(d % 2) * SKV + hi]

            def xq_ap(d, lo, hi):
                return xq[d // 2][:, (d % 2) * SQ + lo:(d % 2) * SQ + hi]

            # ---- K projection (2 rounds over 4 PSUM banks) ------------
            with tc.tile_pool(name="psA", bufs=1, space="PSUM") as psA:
                for rnd in range(2):
                    pk = {}
                    for p in range(2):
                        for qh in range(2):
                            pk[p, qh] = psA.tile([128, 512], F32, tag="pk",
                                                 bufs=4, name=f"pk{rnd}{p}{qh}")
                    for d in range(ND):
                        for p in range(2):
                            for qh in range(2):
                                qc = rnd * 2 + qh
                                nc.tensor.matmul(
                                    pk[p, qh][:],
                                    wk_sb[:, d * GC + p * 128:d * GC + (p + 1) * 128],
                                    xkv_ap(d, qc * 512, (qc + 1) * 512),
                                    start=(d == 0), stop=(d == ND - 1),
                                )
                    for p in range(2):
                        for qh in range(2):
                            qc = rnd * 2 + qh
                            nc.vector.tensor_scalar_add(
                                kt_sb[:, p * SKV + qc * 512:p * SKV + (qc + 1) * 512],
                                pk[p, qh][:], bk_sb[:, p:p + 1])

                # ---- Q projection, qq0 quarter only ------------------
                def emit_qproj_qc_mm(qc, d, pq):
                    for p in range(2):
                        nc.tensor.matmul(
                            pq[p][:],
                            wq_sb[:, d * GC + p * 128:d * GC + (p + 1) * 128],
                            xq_ap(d, qc * 512, (qc + 1) * 512),
                            start=(d == 0), stop=(d == ND - 1),
                        )

                def emit_qproj_qc_add(qc, pq):
                    for p in range(2):
                        nc.vector.tensor_scalar_add(
                            qt_sb[:, p * SQ + qc * 512:p * SQ + (qc + 1) * 512],
                            pq[p][:], bq_sb[:, p:p + 1])

                pq0 = {p: psA.tile([128, 512], F32, tag="pk", bufs=4,
                                   name=f"pq0{p}") for p in range(2)}
                for d in range(ND):
                    emit_qproj_qc_mm(0, d, pq0)
                emit_qproj_qc_add(0, pq0)

            # ---- attention (one-block AV-shift pipeline) --------------
            with tc.tile_pool(name="psC", bufs=1, space="PSUM") as psC:
                blocks = [(qq, t) for qq in range(4) for t in range(2)]
                pt_store = {}
                o_pair = {}
                pending_norm = []

                def flush_norm():
                    while pending_norm:
                        pending_norm.pop(0)()

                def emit_score_exp(k, j):
                    qq, t = blocks[k]
                    st = psC.tile([128, 1024], F32, tag="st2", bufs=2,
                                  name=f"st{k}{j}")
                    # two heads on disjoint PE row groups, concurrent
                    for hp in range(2):
                        nc.tensor.matmul(
                            st[:, hp * 512:(hp + 1) * 512],
                            kt_sb[hp * 64:(hp + 1) * 64,
                                  t * SKV + j * 128:t * SKV + (j + 1) * 128],
                            qt_sb[hp * 64:(hp + 1) * 64,
                                  t * SQ + qq * 512:t * SQ + (qq + 1) * 512],
                            start=True, stop=True,
                        )
                    p_t = at.tile([128, 1024], BF16, tag="pt",
                                  bufs=20, name=f"pt{k}{j}")
                    nc.scalar.activation(p_t[:], st[:], AF.Exp, scale=scale)
                    pt_store[k, j] = p_t

                def emit_av(k, j):
                    qq, t = blocks[k]
                    if k not in o_pair:
                        tag = "aux" if k == 7 else "o_ps"
                        o_pair[k] = {
                            hp: psC.tile([128, 512], F32, tag=tag, bufs=2,
                                         name=f"ops{k}{hp}")
                            for hp in range(2)}
                    p_t = pt_store.pop((k, j))
                    for hp in range(2):
                        h = 2 * t + hp
                        nc.tensor.matmul(
                            o_pair[k][hp][:],
                            vp_sb[:, j * VW + h * 65:j * VW + h * 65 + 128],
                            p_t[:, hp * 512:(hp + 1) * 512],
                            start=(j == 0), stop=(j == NJ - 1),
                        )

                def emit_norm(k):
                    # AV(k) fully emitted: stage O'+rowsum, defer normalize
                    qq, t = blocks[k]
                    for hp in range(2):
                        ot = at.tile([64, 512], F32, tag="ot", bufs=4,
                                     name=f"ot{k}{hp}")
                        nc.vector.tensor_copy(ot[:], o_pair[k][hp][0:64, :])
                        rs = at.tile([1, 512], F32, tag="rs", bufs=4,
                                     name=f"rs{k}{hp}")
                        nc.vector.tensor_copy(rs[:], o_pair[k][hp][64:65, :])

                        def norm(qq=qq, t=t, hp=hp, ot=ot, rs=rs):
                            rcp = at.tile([1, 512], F32, tag="rcp", bufs=4,
                                          name=f"rcp{qq}{t}{hp}")
                            nc.vector.reciprocal_approx_fast(
                                out=rcp[:], in_=rs[:])
                            bcs = at.tile([64, 512], F32, tag="bcs", bufs=4,
                                          name=f"bcs{qq}{t}{hp}")
                            nc.gpsimd.partition_broadcast(
                                bcs[:], rcp[:], channels=64)
                            o_half = o_sbA if qq < 2 else o_sbB
                            col = t * 1024 + (qq % 2) * 512
                            nc.vector.tensor_tensor(
                                out=o_half[hp * 64:(hp + 1) * 64,
                                           col:col + 512],
                                in0=ot[:], in1=bcs[:], op=MUL)

                        pending_norm.append(norm)

                def emit_vproj(j):
                    pv = psC.tile([128, 512], F32, tag="aux", bufs=2,
                                  name=f"pv{j}")
                    for d in range(ND):
                        nc.tensor.matmul(
                            pv[:, 0:VW],
                            xkv_ap(d, j * 128, (j + 1) * 128),
                            wvp_sb[:, d * VW:(d + 1) * VW],
                            start=(d == 0), stop=(d == ND - 1),
                        )
                    nc.vector.tensor_copy(vp_sb[:, j * VW:(j + 1) * VW],
                                          pv[:, 0:VW])
                    nc.gpsimd.memset(
                        vp_sb[:, j * VW + 64:(j + 1) * VW:65], 1.0)

                def emit_outproj_tile(s, n2):
                    po = psC.tile([128, 512], F32, tag="aux", bufs=2,
                                  name=f"po{s}{n2}")
                    o_half = o_sbA if s < 8 else o_sbB
                    s8 = s % 8
                    for tt in range(2):
                        nc.tensor.matmul(
                            po[:],
                            o_half[:, tt * 1024 + s8 * 128:
                                   tt * 1024 + (s8 + 1) * 128],
                            wo_sb[:, tt * D + n2 * 512:tt * D + n2 * 512 + 512],
                            start=(tt == 0), stop=(tt == 1),
                        )
                    ob = at.tile([128, 512], F32, tag="ob", bufs=3,
                                 name=f"ob{s}{n2}")
                    nc.vector.tensor_copy(ob[:], po[:])
                    nc.sync.dma_start(
                        out=out_d[s * 128:(s + 1) * 128,
                                  n2 * 512:(n2 + 1) * 512],
                        in_=ob[:])

                # filler plans: {block: {j: [thunks]}}
                fillers = {k: {} for k in range(8)}
                # block 0: V' projection, one kv tile per j
                for j in range(NJ):
                    fillers[0][j] = [lambda j=j: emit_vproj(j)]
                # blocks 1-2: Q projection qc1-3 (d-pair per slot) + adds
                pq_w = {}

                def qproj_slot(qc, dd):
                    if dd == 0:
                        pq_w[qc] = {p: psC.tile([128, 512], F32, tag="aux",
                                                bufs=2, name=f"pqw{qc}{p}")
                                    for p in range(2)}
                    emit_qproj_qc_mm(qc, 2 * dd, pq_w[qc])
                    emit_qproj_qc_mm(qc, 2 * dd + 1, pq_w[qc])
                    if dd == 3:
                        emit_qproj_qc_add(qc, pq_w[qc])
                        del pq_w[qc]

                slot = 0
                for qc in (1, 2, 3):
                    for dd in range(4):
                        blk = 1 + slot // 16
                        fillers[blk].setdefault(slot % 16, []).append(
                            lambda qc=qc, dd=dd: qproj_slot(qc, dd))
                        slot += 2   # one d-pair every other j
                # blocks 3/5: output projection of qq0/qq1 at js 2..9
                for blk, qq in ((3, 0), (5, 1)):
                    for i in range(4):
                        for n2 in range(2):
                            fillers[blk].setdefault(2 + i * 2 + n2, []).append(
                                lambda s=qq * 4 + i, n2=n2:
                                emit_outproj_tile(s, n2))
                # block 7: its own AVs ride along (aux o_ps pair)
                for j in range(1, NJ):
                    fillers[7].setdefault(j, []).append(
                        lambda j=j - 1: emit_av(7, j))

                for k in range(8):
                    for j in range(NJ):
                        emit_score_exp(k, j)
                        if k >= 1:
                            emit_av(k - 1, j)
                        for thunk in fillers[k].get(j, ()):
                            thunk()
                        if j == 1:
                            flush_norm()
                    if k >= 1:
                        emit_norm(k - 1)

                # ---- drain ------------------------------------------------
                emit_av(7, NJ - 1)
                emit_norm(7)
                flush_norm()
                for s in range(8, 16):
                    for n2 in range(2):
                        emit_outproj_tile(s, n2)

                if _DBG:
                    nc.sync.dma_start(out=dbg_kt[:], in_=kt_sb[:])
                    nc.sync.dma_start(out=dbg_qt[:], in_=qt_sb[:])
                    nc.sync.dma_start(out=dbg_vp[:], in_=vp_sb[:])
                    nc.sync.dma_start(out=dbg_oA[:], in_=o_sbA[:])
                    nc.sync.dma_start(out=dbg_oB[:], in_=o_sbB[:])

    nc.compile()
    return nc


def build_in_maps(inputs):
    query_input = np.asarray(inputs["query_input"], dtype=np.float32)
    kv_input = np.asarray(inputs["kv_input"], dtype=np.float32)
    Wq = np.asarray(inputs["Wq"], dtype=np.float32)
    bq = np.asarray(inputs["bq"], dtype=np.float32)
    Wkv = np.asarray(inputs["Wkv"], dtype=np.float32)
    bkv = np.asarray(inputs["bkv"], dtype=np.float32)
    Wo = np.asarray(inputs["Wo"], dtype=np.float32)

    Wk = Wkv[:, :D]
    Wv = Wkv[:, D:]
    bk = bkv[:D]
    ones64 = np.ones((1, 128), np.float32)

    xT = [np.ascontiguousarray(query_input[b].T).astype(BF16NP) for b in range(B)]
    kvT = [np.ascontiguousarray(kv_input[b].T).astype(BF16NP) for b in range(B)]

    in_maps = []
    for c in range(N_CORES):
        b, g = divmod(c, G)
        c0 = g * GC
        wvp = np.zeros((D, HPG * 65), np.float32)
        for h in range(HPG):
                wvp[:, h * 65:h * 65 + 64] = Wv[:, c0 + h * DH:c0 + (h + 1) * DH]
        bq2 = bq[c0:c0 + GC].reshape(2, 128).T.copy()
        bk2 = bk[c0:c0 + GC].reshape(2, 128).T.copy()
        in_maps.append({
                "xqT": xT[b],
                "xkvT": kvT[b],
                "wq": np.ascontiguousarray(Wq[:, c0:c0 + GC]).astype(BF16NP),
                "wk": np.ascontiguousarray(Wk[:, c0:c0 + GC]).astype(BF16NP),
                "wvp": wvp.astype(BF16NP),
                "wo": np.ascontiguousarray(Wo[c0:c0 + GC, :]).astype(BF16NP),
                "bq2": np.ascontiguousarray(bq2),
                "bk2": np.ascontiguousarray(bk2),
                "ones64": ones64,
        })
    return in_maps


def kernel(query_input, kv_input, Wq, bq, Wkv, bkv, Wo, bo):
    global _nc_cache
    from concourse import bass_utils

    if _nc_cache is None:
        _nc_cache = _build_nc()
    nc = _nc_cache

    Wkv = np.asarray(Wkv, dtype=np.float32)
    Wo = np.asarray(Wo, dtype=np.float32)
    bo = np.asarray(bo, dtype=np.float32)
    bv = np.asarray(bkv, np.float32)[D:]

    in_maps = build_in_maps(dict(
        query_input=query_input, kv_input=kv_input, Wq=Wq, bq=bq,
        Wkv=Wkv, bkv=bkv, Wo=Wo))

    res = bass_utils.run_bass_kernel_spmd(nc, in_maps,
                                          core_ids=list(range(N_CORES)))

    # gather: sum the 4 head-group partials per batch; add biases the device
    # left out (bo, and bv which passes through Wo since softmax rows sum to 1)
    tail = bv @ Wo + bo
    out = np.empty((B, SQ, D), np.float32)
    for b in range(B):
        acc = res.results[b * G + 0]["out_p"].astype(np.float32).copy()
        for g in range(1, G):
                acc += res.results[b * G + g]["out_p"]
        out[b] = acc + tail[None, :]
    return out


# revision 13
# speedup vs baseline: 1.1644x; 1.0237x over previous
"""Multi-head cross attention on 8 Trainium2 NeuronCores.

Sharding: core c = b*4 + g handles batch b (of 2) and head-group g (4 heads
of the 16).  Each core projects Q/K/V for its 4 heads, runs attention, and
computes a partial output projection with its 256 rows of Wo; the host sums
the 4 partials per batch (plus bo and the bv@Wo term, exact because softmax
rows sum to 1).

Schedule (v3): the ACT engine's exp over 16.8M score elements (~147us at
1 elem/cyc/lane) is the per-core bottleneck; the whole schedule exists to
start that exp stream early and never let it gap:
  - inputs stream as paired-d tiles (xkv on the fast sync DMA queue, xq on
    gpsimd); K projection runs 2 rounds over 4 PSUM banks as tiles land,
    then only the qq0 quarter of the Q projection -> first exp at ~24us
  - attention uses a one-block software pipeline: block k emits scores+exps
    for block k and the AV matmuls of block k-1 (p_t tiles carry over), so
    exp never waits on the AV/normalize chain at block boundaries
  - the PE's per-j slack under ACT (~0.5us) is packed with fillers:
    V' projection in block 0, Q projection qc1-3 in blocks 1-2, the output
    projection of finished qq groups in blocks 3/5, block 7's own AVs in
    block 7; outproj(qq2)+(qq3) drain at the end
  - ACT does exp ONLY (table preloaded via a dummy exp at t=0); KT/QT bias
    adds run on DVE, V'-ones fixups on GpSimd memset, softmax normalize is
    reciprocal_approx_fast + partition-broadcast + DVE multiply
PSUM: st 2x[128,1024] (4 banks) + o_ps pair (2) + aux (2) shared serially
by {pv, woven pq, po, block-7 o_ps pair}.
Dataflow is fully transposed (see build_in_maps): QT/KT = W.T @ xT,
V' = xkvT.T @ Wv' with a ones column per head so the AV matmul also emits
softmax row-sums.  Matmuls in bf16, fp32 PSUM accumulation.
"""

import sys

sys.path.insert(0, "/opt/trn_rl_repo")

import ml_dtypes
import numpy as np

BF16NP = ml_dtypes.bfloat16

B, SQ, SKV, D, H = 2, 2048, 2048, 1024, 16
DH = D // H          # 64
N_CORES = 8
G = 4                # head groups
HPG = H // G         # heads per group = 4
GC = HPG * DH        # group width = 256

_nc_cache = None


def _build_nc():
    import concourse.mybir as mybir
    import concourse.tile as tile
    from concourse import bacc

    F32 = mybir.dt.float32
    BF16 = mybir.dt.bfloat16
    AF = mybir.ActivationFunctionType
    MUL = mybir.AluOpType.mult

    nc = bacc.Bacc("TRN2", target_bir_lowering=False, debug=False,
                   num_devices=N_CORES)

    xqT_d = nc.dram_tensor("xqT", [D, SQ], BF16, kind="ExternalInput").ap()
    xkvT_d = nc.dram_tensor("xkvT", [D, SKV], BF16, kind="ExternalInput").ap()
    wq_d = nc.dram_tensor("wq", [D, GC], BF16, kind="ExternalInput").ap()
    wk_d = nc.dram_tensor("wk", [D, GC], BF16, kind="ExternalInput").ap()
    # Wv' with a zero column after each head's 64 (slots for the ones column)
    wvp_d = nc.dram_tensor("wvp", [D, HPG * 65], BF16, kind="ExternalInput").ap()
    wo_d = nc.dram_tensor("wo", [GC, D], BF16, kind="ExternalInput").ap()
    bq_d = nc.dram_tensor("bq2", [128, 2], F32, kind="ExternalInput").ap()
    bk_d = nc.dram_tensor("bk2", [128, 2], F32, kind="ExternalInput").ap()
    # legacy input, unused on-device but still part of the host contract
    nc.dram_tensor("ones64", [1, 128], mybir.dt.float32r, kind="ExternalInput")
    out_d = nc.dram_tensor("out_p", [SQ, D], F32, kind="ExternalOutput").ap()
    _DBG = bool(globals().get("_DEBUG_DUMPS"))
    if _DBG:
        dbg_kt = nc.dram_tensor("dbg_kt", [128, 2 * SKV], BF16,
                                kind="ExternalOutput").ap()
        dbg_qt = nc.dram_tensor("dbg_qt", [128, 2 * SQ], BF16,
                                kind="ExternalOutput").ap()
        dbg_vp = nc.dram_tensor("dbg_vp", [128, (SKV // 128) * HPG * 65 + 63],
                                BF16, kind="ExternalOutput").ap()
        dbg_oA = nc.dram_tensor("dbg_oA", [128, 2 * 1024], BF16,
                                kind="ExternalOutput").ap()
        dbg_oB = nc.dram_tensor("dbg_oB", [128, 2 * 1024], BF16,
                                kind="ExternalOutput").ap()

    ND = D // 128        # 8 d-tiles (contraction over D)
    NP = ND // 2         # 4 paired-d input tiles
    NJ = SKV // 128      # 16 kv tiles
    VW = HPG * 65        # 260, V' row width
    scale = 1.0 / float(np.sqrt(DH))

    with tile.TileContext(nc) as tc:
        with (
            tc.tile_pool(name="persist", bufs=1) as pp,
            tc.tile_pool(name="attn", bufs=1) as at,
        ):
            # ---- persistent tiles -------------------------------------
            qt_sb = pp.tile([128, 2 * SQ], BF16, tag="qt_sb")
            kt_sb = pp.tile([128, 2 * SKV], BF16, tag="kt_sb")
            vp_sb = pp.tile([128, NJ * VW + 63], BF16, tag="vp_sb")
            o_sbA = pp.tile([128, 2 * 1024], BF16, tag="o_sbA")
            o_sbB = pp.tile([128, 2 * 1024], BF16, tag="o_sbB")
            bq_sb = pp.tile([128, 2], F32, tag="bq_sb")
            bk_sb = pp.tile([128, 2], F32, tag="bk_sb")
            wk_sb = pp.tile([128, ND * GC], BF16, tag="wk_sb")
            wq_sb = pp.tile([128, ND * GC], BF16, tag="wq_sb")
            wvp_sb = pp.tile([128, ND * VW], BF16, tag="wvp_sb")
            wo_sb = pp.tile([128, 2 * D], BF16, tag="wo_sb")
            warm = pp.tile([1, 32], F32, tag="warm")

            # preload the exp spline tables while DMAs stream
            nc.gpsimd.memset(warm[:], 0.0)
            nc.scalar.activation(warm[:], warm[:], AF.Exp)
            # zero the 63-col tail pad of V' (AV lhsT windows over-read it)
            nc.gpsimd.memset(vp_sb[:, NJ * VW:NJ * VW + 63], 0.0)

            # ---- DMA issue (order = priority) -------------------------
            # sync queue: wk first, then the xkv stream (per-d tiles for
            # fine-grained arrival), then weights needed later
            nc.sync.dma_start(
                out=wk_sb[:].rearrange("p (t n) -> p t n", t=ND),
                in_=wk_d.rearrange("(t p) n -> p t n", p=128))
            xkv = []
            for d in range(ND):
                t = pp.tile([128, SKV], BF16, tag=f"xkv{d}", name=f"xkv{d}")
                nc.sync.dma_start(out=t[:],
                                  in_=xkvT_d[d * 128:(d + 1) * 128, :])
                xkv.append(t)
            nc.sync.dma_start(
                out=wvp_sb[:].rearrange("p (t n) -> p t n", t=ND),
                in_=wvp_d.rearrange("(t p) n -> p t n", p=128))
            nc.sync.dma_start(
                out=wo_sb[:].rearrange("p (t n) -> p t n", t=2),
                in_=wo_d.rearrange("(t p) n -> p t n", p=128))
            # gpsimd queue: biases + wq lead, then the xq stream
            nc.gpsimd.dma_start(out=bk_sb[:], in_=bk_d[:])
            nc.gpsimd.dma_start(out=bq_sb[:], in_=bq_d[:])
            nc.gpsimd.dma_start(
                out=wq_sb[:].rearrange("p (t n) -> p t n", t=ND),
                in_=wq_d.rearrange("(t p) n -> p t n", p=128))
            xq = []
            for d in range(ND):
                t = pp.tile([128, SQ], BF16, tag=f"xq{d}", name=f"xq{d}")
                nc.gpsimd.dma_start(out=t[:],
                                    in_=xqT_d[d * 128:(d + 1) * 128, :])
                xq.append(t)

            def xkv_ap(d, lo, hi):
                return xkv[d][:, lo:hi]

            def xq_ap(d, lo, hi):
                return xq[d][:, lo:hi]

            # ---- K projection (2 rounds over 4 PSUM banks) ------------
            with tc.tile_pool(name="psA", bufs=1, space="PSUM") as psA:
                for rnd in range(2):
                    pk = {}
                    for p in range(2):
                        for qh in range(2):
                            pk[p, qh] = psA.tile([128, 512], F32, tag="pk",
                                                 bufs=4, name=f"pk{rnd}{p}{qh}")
                    for d in range(ND):
                        for p in range(2):
                            for qh in range(2):
                                qc = rnd * 2 + qh
                                nc.tensor.matmul(
                                    pk[p, qh][:],
                                    wk_sb[:, d * GC + p * 128:d * GC + (p + 1) * 128],
                                    xkv_ap(d, qc * 512, (qc + 1) * 512),
                                    start=(d == 0), stop=(d == ND - 1),
                                )
                    for p in range(2):
                        for qh in range(2):
                            qc = rnd * 2 + qh
                            nc.vector.tensor_scalar_add(
                                kt_sb[:, p * SKV + qc * 512:p * SKV + (qc + 1) * 512],
                                pk[p, qh][:], bk_sb[:, p:p + 1])

                # ---- Q projection, qq0 quarter only ------------------
                def emit_qproj_qc_mm(qc, d, pq):
                    for p in range(2):
                        nc.tensor.matmul(
                            pq[p][:],
                            wq_sb[:, d * GC + p * 128:d * GC + (p + 1) * 128],
                            xq_ap(d, qc * 512, (qc + 1) * 512),
                            start=(d == 0), stop=(d == ND - 1),
                        )

                def emit_qproj_qc_add(qc, pq):
                    for p in range(2):
                        nc.vector.tensor_scalar_add(
                            qt_sb[:, p * SQ + qc * 512:p * SQ + (qc + 1) * 512],
                            pq[p][:], bq_sb[:, p:p + 1])

                pq0 = {p: psA.tile([128, 512], F32, tag="pk", bufs=4,
                                   name=f"pq0{p}") for p in range(2)}
                for d in range(ND):
                    emit_qproj_qc_mm(0, d, pq0)
                emit_qproj_qc_add(0, pq0)

            # ---- attention (one-block AV-shift pipeline) --------------
            with tc.tile_pool(name="psC", bufs=1, space="PSUM") as psC:
                blocks = [(qq, t) for qq in range(4) for t in range(2)]
                pt_store = {}
                o_pair = {}
                pending_norm = []

                def flush_norm():
                    while pending_norm:
                        pending_norm.pop(0)()

                def emit_score_exp(k, j):
                    qq, t = blocks[k]
                    st = psC.tile([128, 1024], F32, tag="st2", bufs=2,
                                  name=f"st{k}{j}")
                    # two heads on disjoint PE row groups, concurrent
                    for hp in range(2):
                        nc.tensor.matmul(
                            st[:, hp * 512:(hp + 1) * 512],
                            kt_sb[hp * 64:(hp + 1) * 64,
                                  t * SKV + j * 128:t * SKV + (j + 1) * 128],
                            qt_sb[hp * 64:(hp + 1) * 64,
                                  t * SQ + qq * 512:t * SQ + (qq + 1) * 512],
                            start=True, stop=True,
                        )
                    p_t = at.tile([128, 1024], BF16, tag="pt",
                                  bufs=20, name=f"pt{k}{j}")
                    nc.scalar.activation(p_t[:], st[:], AF.Exp, scale=scale)
                    pt_store[k, j] = p_t

                def emit_av(k, j):
                    qq, t = blocks[k]
                    if k not in o_pair:
                        tag = "aux" if k == 7 else "o_ps"
                        o_pair[k] = {
                            hp: psC.tile([128, 512], F32, tag=tag, bufs=2,
                                         name=f"ops{k}{hp}")
                            for hp in range(2)}
                    p_t = pt_store.pop((k, j))
                    for hp in range(2):
                        h = 2 * t + hp
                        nc.tensor.matmul(
                            o_pair[k][hp][:],
                            vp_sb[:, j * VW + h * 65:j * VW + h * 65 + 128],
                            p_t[:, hp * 512:(hp + 1) * 512],
                            start=(j == 0), stop=(j == NJ - 1),
                        )

                def emit_norm(k):
                    # AV(k) fully emitted: stage O'+rowsum, defer normalize
                    qq, t = blocks[k]
                    for hp in range(2):
                        ot = at.tile([64, 512], F32, tag="ot", bufs=4,
                                     name=f"ot{k}{hp}")
                        nc.vector.tensor_copy(ot[:], o_pair[k][hp][0:64, :])
                        rs = at.tile([1, 512], F32, tag="rs", bufs=4,
                                     name=f"rs{k}{hp}")
                        nc.vector.tensor_copy(rs[:], o_pair[k][hp][64:65, :])

                        def norm(qq=qq, t=t, hp=hp, ot=ot, rs=rs):
                            rcp = at.tile([1, 512], F32, tag="rcp", bufs=4,
                                          name=f"rcp{qq}{t}{hp}")
                            nc.vector.reciprocal_approx_fast(
                                out=rcp[:], in_=rs[:])
                            bcs = at.tile([64, 512], F32, tag="bcs", bufs=4,
                                          name=f"bcs{qq}{t}{hp}")
                            nc.gpsimd.partition_broadcast(
                                bcs[:], rcp[:], channels=64)
                            o_half = o_sbA if qq < 2 else o_sbB
                            col = t * 1024 + (qq % 2) * 512
                            nc.vector.tensor_tensor(
                                out=o_half[hp * 64:(hp + 1) * 64,
                                           col:col + 512],
                                in0=ot[:], in1=bcs[:], op=MUL)

                        pending_norm.append(norm)

                def emit_vproj(j):
                    pv = psC.tile([128, 512], F32, tag="aux", bufs=2,
                                  name=f"pv{j}")
                    for d in range(ND):
                        nc.tensor.matmul(
                            pv[:, 0:VW],
                            xkv_ap(d, j * 128, (j + 1) * 128),
                            wvp_sb[:, d * VW:(d + 1) * VW],
                            start=(d == 0), stop=(d == ND - 1),
                        )
                    nc.vector.tensor_copy(vp_sb[:, j * VW:(j + 1) * VW],
                                          pv[:, 0:VW])
                    nc.gpsimd.memset(
                        vp_sb[:, j * VW + 64:(j + 1) * VW:65], 1.0)

                ob_group = {}

                def emit_outproj_tile(s, n2):
                    po = psC.tile([128, 512], F32, tag="aux", bufs=2,
                                  name=f"po{s}{n2}")
                    o_half = o_sbA if s < 8 else o_sbB
                    s8 = s % 8
                    for tt in range(2):
                        nc.tensor.matmul(
                            po[:],
                            o_half[:, tt * 1024 + s8 * 128:
                                   tt * 1024 + (s8 + 1) * 128],
                            wo_sb[:, tt * D + n2 * 512:tt * D + n2 * 512 + 512],
                            start=(tt == 0), stop=(tt == 1),
                        )
                    g = s // 4
                    if (g, n2) not in ob_group:
                        ob_group[g, n2] = at.tile([128, 4 * 512], F32,
                                                  tag="ob4", bufs=2,
                                                  name=f"ob4_{g}{n2}")
                    nc.vector.tensor_copy(
                        ob_group[g, n2][:, (s % 4) * 512:(s % 4 + 1) * 512],
                        po[:])

                def emit_out_dma(g, n2):
                    # one descriptor for 4 q-tiles x 512 cols
                    ob4 = ob_group.pop((g, n2))
                    nc.sync.dma_start(
                        out=out_d[g * 512:(g + 1) * 512,
                                  n2 * 512:(n2 + 1) * 512].rearrange(
                                      "(t p) n -> p t n", p=128),
                        in_=ob4[:].rearrange("p (t n) -> p t n", t=4))

                # filler plans: {block: {j: [thunks]}}
                fillers = {k: {} for k in range(8)}
                # block 0: V' projection, one kv tile per j
                for j in range(NJ):
                    fillers[0][j] = [lambda j=j: emit_vproj(j)]
                # blocks 1-2: Q projection qc1-3 (d-pair per slot) + adds
                pq_w = {}

                def qproj_slot(qc, dd):
                    if dd == 0:
                        pq_w[qc] = {p: psC.tile([128, 512], F32, tag="aux",
                                                bufs=2, name=f"pqw{qc}{p}")
                                    for p in range(2)}
                    emit_qproj_qc_mm(qc, 2 * dd, pq_w[qc])
                    emit_qproj_qc_mm(qc, 2 * dd + 1, pq_w[qc])
                    if dd == 3:
                        emit_qproj_qc_add(qc, pq_w[qc])
                        del pq_w[qc]

                slot = 0
                for qc in (1, 2, 3):
                    for dd in range(4):
                        blk = 1 + slot // 16
                        fillers[blk].setdefault(slot % 16, []).append(
                            lambda qc=qc, dd=dd: qproj_slot(qc, dd))
                        slot += 2   # one d-pair every other j
                # blocks 3/5: output projection of qq0/qq1 at js 2..11
                for blk, qq in ((3, 0), (5, 1)):
                    for n2 in range(2):
                        for i in range(4):
                            fillers[blk].setdefault(2 + n2 * 5 + i, []).append(
                                lambda s=qq * 4 + i, n2=n2:
                                emit_outproj_tile(s, n2))
                        fillers[blk].setdefault(6 + n2 * 5, []).append(
                            lambda g=qq, n2=n2: emit_out_dma(g, n2))
                # block 7: its own AVs ride along (aux o_ps pair)
                for j in range(1, NJ):
                    fillers[7].setdefault(j, []).append(
                        lambda j=j - 1: emit_av(7, j))

                for k in range(8):
                    for j in range(NJ):
                        emit_score_exp(k, j)
                        # AVs of the previous block, one slot behind so the
                        # o_ps pair WAR-wait hides under an exp period
                        if k >= 1 and j >= 1:
                            emit_av(k - 1, j - 1)
                        for thunk in fillers[k].get(j, ()):
                            thunk()
                        if j == 1:
                            flush_norm()
                    if k >= 1:
                        emit_av(k - 1, NJ - 1)
                        emit_norm(k - 1)

                # ---- drain ------------------------------------------------
                emit_av(7, NJ - 1)
                emit_norm(7)
                flush_norm()
                for g in (2, 3):
                    for n2 in range(2):
                        for i in range(4):
                            emit_outproj_tile(g * 4 + i, n2)
                        emit_out_dma(g, n2)

                if _DBG:
                    nc.sync.dma_start(out=dbg_kt[:], in_=kt_sb[:])
                    nc.sync.dma_start(out=dbg_qt[:], in_=qt_sb[:])
                    nc.sync.dma_start(out=dbg_vp[:], in_=vp_sb[:])
                    nc.sync.dma_start(out=dbg_oA[:], in_=o_sbA[:])
                    nc.sync.dma_start(out=dbg_oB[:], in_=o_sbB[:])

    nc.compile()
    return nc


def build_in_maps(inputs):
    query_input = np.asarray(inputs["query_input"], dtype=np.float32)
    kv_input = np.asarray(inputs["kv_input"], dtype=np.float32)
    Wq = np.asarray(inputs["Wq"], dtype=np.float32)
    bq = np.asarray(inputs["bq"], dtype=np.float32)
    Wkv = np.asarray(inputs["Wkv"], dtype=np.float32)
    bkv = np.asarray(inputs["bkv"], dtype=np.float32)
    Wo = np.asarray(inputs["Wo"], dtype=np.float32)

    Wk = Wkv[:, :D]
    Wv = Wkv[:, D:]
    bk = bkv[:D]
    ones64 = np.ones((1, 128), np.float32)

    xT = [np.ascontiguousarray(query_input[b].T).astype(BF16NP) for b in range(B)]
    kvT = [np.ascontiguousarray(kv_input[b].T).astype(BF16NP) for b in range(B)]

    in_maps = []
    for c in range(N_CORES):
        b, g = divmod(c, G)
        c0 = g * GC
        wvp = np.zeros((D, HPG * 65), np.float32)
        for h in range(HPG):
                wvp[:, h * 65:h * 65 + 64] = Wv[:, c0 + h * DH:c0 + (h + 1) * DH]
        bq2 = bq[c0:c0 + GC].reshape(2, 128).T.copy()
        bk2 = bk[c0:c0 + GC].reshape(2, 128).T.copy()
        in_maps.append({
                "xqT": xT[b],
                "xkvT": kvT[b],
                "wq": np.ascontiguousarray(Wq[:, c0:c0 + GC]).astype(BF16NP),
                "wk": np.ascontiguousarray(Wk[:, c0:c0 + GC]).astype(BF16NP),
                "wvp": wvp.astype(BF16NP),
                "wo": np.ascontiguousarray(Wo[c0:c0 + GC, :]).astype(BF16NP),
                "bq2": np.ascontiguousarray(bq2),
                "bk2": np.ascontiguousarray(bk2),
                "ones64": ones64,
        })
    return in_maps


def kernel(query_input, kv_input, Wq, bq, Wkv, bkv, Wo, bo):
    global _nc_cache
    from concourse import bass_utils

    if _nc_cache is None:
        _nc_cache = _build_nc()
    nc = _nc_cache

    Wkv = np.asarray(Wkv, dtype=np.float32)
    Wo = np.asarray(Wo, dtype=np.float32)
    bo = np.asarray(bo, dtype=np.float32)
    bv = np.asarray(bkv, np.float32)[D:]

    in_maps = build_in_maps(dict(
        query_input=query_input, kv_input=kv_input, Wq=Wq, bq=bq,
        Wkv=Wkv, bkv=bkv, Wo=Wo))

    res = bass_utils.run_bass_kernel_spmd(nc, in_maps,
                                          core_ids=list(range(N_CORES)))

    # gather: sum the 4 head-group partials per batch; add biases the device
    # left out (bo, and bv which passes through Wo since softmax rows sum to 1)
    tail = bv @ Wo + bo
    out = np.empty((B, SQ, D), np.float32)
    for b in range(B):
        acc = res.results[b * G + 0]["out_p"].astype(np.float32).copy()
        for g in range(1, G):
                acc += res.results[b * G + g]["out_p"]
        out[b] = acc + tail[None, :]
    return out


# revision 26
# speedup vs baseline: 1.1824x; 1.0154x over previous
"""Multi-head cross attention on 8 Trainium2 NeuronCores.

Sharding: core c = b*4 + g handles batch b (of 2) and head-group g (4 heads
of the 16).  Each core projects Q/K/V for its 4 heads, runs attention, and
computes a partial output projection with its 256 rows of Wo; the host sums
the 4 partials per batch (plus bo and the bv@Wo term, exact because softmax
rows sum to 1).

Schedule (v3): the ACT engine's exp over 16.8M score elements (~147us at
1 elem/cyc/lane) is the per-core bottleneck; the whole schedule exists to
start that exp stream early and never let it gap:
  - inputs stream as paired-d tiles (xkv on the fast sync DMA queue, xq on
    gpsimd); K projection runs 2 rounds over 4 PSUM banks as tiles land,
    then only the qq0 quarter of the Q projection -> first exp at ~24us
  - attention uses a one-block software pipeline: block k emits scores+exps
    for block k and the AV matmuls of block k-1 (p_t tiles carry over), so
    exp never waits on the AV/normalize chain at block boundaries
  - the PE's per-j slack under ACT (~0.5us) is packed with fillers:
    V' projection in block 0, Q projection qc1-3 in blocks 1-2, the output
    projection of finished qq groups in blocks 3/5, block 7's own AVs in
    block 7; outproj(qq2)+(qq3) drain at the end
  - ACT does exp ONLY (table preloaded via a dummy exp at t=0); KT/QT bias
    adds run on DVE, V'-ones fixups on GpSimd memset, softmax normalize is
    reciprocal_approx_fast + partition-broadcast + DVE multiply
PSUM: st 2x[128,1024] (4 banks) + o_ps pair (2) + aux (2) shared serially
by {pv, woven pq, po, block-7 o_ps pair}.
Dataflow is fully transposed (see build_in_maps): QT/KT = W.T @ xT,
V' = xkvT.T @ Wv' with a ones column per head so the AV matmul also emits
softmax row-sums.  Matmuls in bf16, fp32 PSUM accumulation.
"""

import sys

sys.path.insert(0, "/opt/trn_rl_repo")

import ml_dtypes
import numpy as np

BF16NP = ml_dtypes.bfloat16

B, SQ, SKV, D, H = 2, 2048, 2048, 1024, 16
DH = D // H          # 64
N_CORES = 8
G = 4                # head groups
HPG = H // G         # heads per group = 4
GC = HPG * DH        # group width = 256

_nc_cache = None


def _build_nc():
    import concourse.mybir as mybir
    import concourse.tile as tile
    from concourse import bacc

    F32 = mybir.dt.float32
    BF16 = mybir.dt.bfloat16
    AF = mybir.ActivationFunctionType
    MUL = mybir.AluOpType.mult

    nc = bacc.Bacc("TRN2", target_bir_lowering=False, debug=False,
                   num_devices=N_CORES)

    xqT_d = nc.dram_tensor("xqT", [D, SQ], BF16, kind="ExternalInput").ap()
    xkvT_d = nc.dram_tensor("xkvT", [D, SKV], BF16, kind="ExternalInput").ap()
    wq_d = nc.dram_tensor("wq", [D, GC], BF16, kind="ExternalInput").ap()
    wk_d = nc.dram_tensor("wk", [D, GC], BF16, kind="ExternalInput").ap()
    # Wv' with a zero column after each head's 64 (slots for the ones column)
    wvp_d = nc.dram_tensor("wvp", [D, HPG * 65], BF16, kind="ExternalInput").ap()
    wo_d = nc.dram_tensor("wo", [GC, D], BF16, kind="ExternalInput").ap()
    bq_d = nc.dram_tensor("bq2", [128, 2], F32, kind="ExternalInput").ap()
    bk_d = nc.dram_tensor("bk2", [128, 2], F32, kind="ExternalInput").ap()
    # legacy input, unused on-device but still part of the host contract
    nc.dram_tensor("ones64", [1, 128], mybir.dt.float32r, kind="ExternalInput")
    out_d = nc.dram_tensor("out_p", [SQ, D], F32, kind="ExternalOutput").ap()
    _DBG = bool(globals().get("_DEBUG_DUMPS"))
    if _DBG:
        dbg_kt = nc.dram_tensor("dbg_kt", [128, 2 * SKV], BF16,
                                kind="ExternalOutput").ap()
        dbg_qt = nc.dram_tensor("dbg_qt", [128, 2 * SQ], BF16,
                                kind="ExternalOutput").ap()
        dbg_vp = nc.dram_tensor("dbg_vp", [128, (SKV // 128) * HPG * 65 + 63],
                                BF16, kind="ExternalOutput").ap()
        dbg_oA = nc.dram_tensor("dbg_oA", [128, 2 * 1024], BF16,
                                kind="ExternalOutput").ap()
        dbg_oB = nc.dram_tensor("dbg_oB", [128, 2 * 1024], BF16,
                                kind="ExternalOutput").ap()

    ND = D // 128        # 8 d-tiles (contraction over D)
    NP = ND // 2         # 4 paired-d input tiles
    NJ = SKV // 128      # 16 kv tiles
    VW = HPG * 65        # 260, V' row width
    scale = 1.0 / float(np.sqrt(DH))

    with tile.TileContext(nc) as tc:
        with (
            tc.tile_pool(name="persist", bufs=1) as pp,
            tc.tile_pool(name="attn", bufs=1) as at,
        ):
            # ---- persistent tiles -------------------------------------
            qt_sb = pp.tile([128, 2 * SQ], BF16, tag="qt_sb")
            kt_sb = pp.tile([128, 2 * SKV], BF16, tag="kt_sb")
            vp_sb = pp.tile([128, NJ * VW + 63], BF16, tag="vp_sb")
            # normalized O, per qq pair/quarter (split tiles so late norm
            # writes don't false-WAR earlier outproj reads)
            o_sbA = pp.tile([128, 2 * 1024], BF16, tag="o_sbA")
            o_sbB2 = pp.tile([128, 2 * 512], BF16, tag="o_sbB2")
            o_sbB3 = pp.tile([128, 2 * 512], BF16, tag="o_sbB3")
            bq_sb = pp.tile([128, 2], F32, tag="bq_sb")
            bk_sb = pp.tile([128, 2], F32, tag="bk_sb")
            wk_sb = pp.tile([128, ND * GC], BF16, tag="wk_sb")
            wq_sb = pp.tile([128, ND * GC], BF16, tag="wq_sb")
            wvp_sb = pp.tile([128, ND * VW], BF16, tag="wvp_sb")
            wo_sb = pp.tile([128, 2 * D], BF16, tag="wo_sb")
            warm = pp.tile([1, 32], F32, tag="warm")

            nc.gpsimd.memset(warm[:], 0.0)
            # zero the 63-col tail pad of V' (AV lhsT windows over-read it)
            nc.gpsimd.memset(vp_sb[:, NJ * VW:NJ * VW + 63], 0.0)

            # ---- DMA issue (3 queues in parallel; order = priority) ----
            # sync queue: wk first, then the xkv stream (per-d tiles)
            nc.sync.dma_start(
                out=wk_sb[:].rearrange("p (t n) -> p t n", t=ND),
                in_=wk_d.rearrange("(t p) n -> p t n", p=128))
            xkv = []
            for d in range(ND):
                t = pp.tile([128, SKV], BF16, tag=f"xkv{d}", name=f"xkv{d}")
                nc.sync.dma_start(out=t[:],
                                  in_=xkvT_d[d * 128:(d + 1) * 128, :])
                xkv.append(t)
            # scalar queue: biases + wq, the xq stream, late weights (the
            # ACT engine is idle until the first exp anyway)
            nc.scalar.dma_start(out=bk_sb[:], in_=bk_d[:])
            nc.scalar.dma_start(out=bq_sb[:], in_=bq_d[:])
            nc.scalar.dma_start(
                out=wq_sb[:].rearrange("p (t n) -> p t n", t=ND),
                in_=wq_d.rearrange("(t p) n -> p t n", p=128))
            xq = []
            for d in range(ND):
                t = pp.tile([128, SQ], BF16, tag=f"xq{d}", name=f"xq{d}")
                nc.scalar.dma_start(out=t[:],
                                    in_=xqT_d[d * 128:(d + 1) * 128, :])
                xq.append(t)
            nc.scalar.dma_start(
                out=wvp_sb[:].rearrange("p (t n) -> p t n", t=ND),
                in_=wvp_d.rearrange("(t p) n -> p t n", p=128))
            nc.scalar.dma_start(
                out=wo_sb[:].rearrange("p (t n) -> p t n", t=2),
                in_=wo_d.rearrange("(t p) n -> p t n", p=128))
            # preload the exp spline tables while DMAs stream
            nc.scalar.activation(warm[:], warm[:], AF.Exp)

            def xkv_ap(d, lo, hi):
                return xkv[d][:, lo:hi]

            def xq_ap(d, lo, hi):
                return xq[d][:, lo:hi]

            # ---- K projection (2 rounds over 4 PSUM banks) ------------
            with tc.tile_pool(name="psA", bufs=1, space="PSUM") as psA:
                for rnd in range(2):
                    pk = {}
                    for p in range(2):
                        for qh in range(2):
                            pk[p, qh] = psA.tile([128, 512], F32, tag="pk",
                                                 bufs=4, name=f"pk{rnd}{p}{qh}")
                    for d in range(ND):
                        for p in range(2):
                            for qh in range(2):
                                qc = rnd * 2 + qh
                                nc.tensor.matmul(
                                    pk[p, qh][:],
                                    wk_sb[:, d * GC + p * 128:d * GC + (p + 1) * 128],
                                    xkv_ap(d, qc * 512, (qc + 1) * 512),
                                    start=(d == 0), stop=(d == ND - 1),
                                )
                    for p in range(2):
                        for qh in range(2):
                            qc = rnd * 2 + qh
                            nc.vector.tensor_scalar_add(
                                kt_sb[:, p * SKV + qc * 512:p * SKV + (qc + 1) * 512],
                                pk[p, qh][:], bk_sb[:, p:p + 1])

            # ---- attention (one-block AV-shift pipeline) --------------
            with tc.tile_pool(name="psC", bufs=1, space="PSUM") as psC:
                blocks = [(qq, t) for qq in range(4) for t in range(2)]
                pt_store = {}
                o_pair = {}
                pending_norm = []

                def o_half_col(qq, t):
                    if qq < 2:
                        return o_sbA, t * 1024 + qq * 512
                    return (o_sbB2 if qq == 2 else o_sbB3), t * 512

                def emit_qproj_qc_mm(qc, d, pq):
                    # pq: {p: AP of a [128,512] fp32 PSUM accumulator}
                    for p in range(2):
                        nc.tensor.matmul(
                            pq[p],
                            wq_sb[:, d * GC + p * 128:d * GC + (p + 1) * 128],
                            xq_ap(d, qc * 512, (qc + 1) * 512),
                            start=(d == 0), stop=(d == ND - 1),
                        )

                def emit_qproj_qc_add(qc, pq):
                    for p in range(2):
                        nc.vector.tensor_scalar_add(
                            qt_sb[:, p * SQ + qc * 512:p * SQ + (qc + 1) * 512],
                            pq[p], bq_sb[:, p:p + 1])

                def flush_norm():
                    while pending_norm:
                        pending_norm.pop(0)()

                def emit_score_exp(k, j):
                    qq, t = blocks[k]
                    st = psC.tile([128, 1024], F32, tag="st2", bufs=2,
                                  name=f"st{k}{j}")
                    # two heads on disjoint PE row groups, concurrent
                    for hp in range(2):
                        nc.tensor.matmul(
                            st[:, hp * 512:(hp + 1) * 512],
                            kt_sb[hp * 64:(hp + 1) * 64,
                                  t * SKV + j * 128:t * SKV + (j + 1) * 128],
                            qt_sb[hp * 64:(hp + 1) * 64,
                                  t * SQ + qq * 512:t * SQ + (qq + 1) * 512],
                            start=True, stop=True,
                        )
                    p_t = at.tile([128, 1024], BF16, tag="pt",
                                  bufs=20, name=f"pt{k}{j}")
                    nc.scalar.activation(p_t[:], st[:], AF.Exp, scale=scale)
                    pt_store[k, j] = p_t

                def emit_av(k, j):
                    qq, t = blocks[k]
                    if k not in o_pair:
                        tag = "aux" if k == 7 else "o_ps"
                        o_pair[k] = {
                            hp: psC.tile([128, 512], F32, tag=tag, bufs=2,
                                         name=f"ops{k}{hp}")
                            for hp in range(2)}
                    p_t = pt_store.pop((k, j))
                    for hp in range(2):
                        h = 2 * t + hp
                        nc.tensor.matmul(
                            o_pair[k][hp][:],
                            vp_sb[:, j * VW + h * 65:j * VW + h * 65 + 128],
                            p_t[:, hp * 512:(hp + 1) * 512],
                            start=(j == 0), stop=(j == NJ - 1),
                        )

                def emit_norm(k):
                    # AV(k) fully emitted: stage O'+rowsum, defer normalize
                    qq, t = blocks[k]
                    for hp in range(2):
                        ot = at.tile([64, 512], F32, tag="ot", bufs=4,
                                     name=f"ot{k}{hp}")
                        nc.vector.tensor_copy(ot[:], o_pair[k][hp][0:64, :])
                        rs = at.tile([1, 512], F32, tag="rs", bufs=4,
                                     name=f"rs{k}{hp}")
                        nc.vector.tensor_copy(rs[:], o_pair[k][hp][64:65, :])

                        def norm(qq=qq, t=t, hp=hp, ot=ot, rs=rs):
                            rcp = at.tile([1, 512], F32, tag="rcp", bufs=4,
                                          name=f"rcp{qq}{t}{hp}")
                            nc.vector.reciprocal_approx_fast(
                                out=rcp[:], in_=rs[:])
                            bcs = at.tile([64, 512], F32, tag="bcs", bufs=4,
                                          name=f"bcs{qq}{t}{hp}")
                            nc.gpsimd.partition_broadcast(
                                bcs[:], rcp[:], channels=64)
                            o_half, col = o_half_col(qq, t)
                            nc.vector.tensor_tensor(
                                out=o_half[hp * 64:(hp + 1) * 64,
                                           col:col + 512],
                                in0=ot[:], in1=bcs[:], op=MUL)

                        pending_norm.append(norm)

                def emit_vproj(j):
                    pv = psC.tile([128, 512], F32, tag="aux", bufs=2,
                                  name=f"pv{j}")
                    for d in range(ND):
                        nc.tensor.matmul(
                            pv[:, 0:VW],
                            xkv_ap(d, j * 128, (j + 1) * 128),
                            wvp_sb[:, d * VW:(d + 1) * VW],
                            start=(d == 0), stop=(d == ND - 1),
                        )
                    nc.vector.tensor_copy(vp_sb[:, j * VW:(j + 1) * VW],
                                          pv[:, 0:VW])
                    nc.gpsimd.memset(
                        vp_sb[:, j * VW + 64:(j + 1) * VW:65], 1.0)

                ob_group = {}

                def emit_outproj_tile(s, n2, drain=False):
                    po = psC.tile([128, 512], F32, tag="aux", bufs=2,
                                  name=f"po{s}{n2}")
                    qq = s // 4
                    for tt in range(2):
                        o_half, col = o_half_col(qq, tt)
                        lhs_lo = col + (s % 4) * 128
                        nc.tensor.matmul(
                            po[:],
                            o_half[:, lhs_lo:lhs_lo + 128],
                            wo_sb[:, tt * D + n2 * 512:tt * D + n2 * 512 + 512],
                            start=(tt == 0), stop=(tt == 1),
                        )
                    g = s // 4
                    if (g, n2) not in ob_group:
                        ob_group[g, n2] = at.tile([128, 4 * 512], F32,
                                                  tag="ob4", bufs=2,
                                                  name=f"ob4_{g}{n2}")
                    dst = ob_group[g, n2][:, (s % 4) * 512:(s % 4 + 1) * 512]
                    if drain:
                        # ACT is idle after the last exp; keep DVE free for
                        # the normalize chains
                        nc.scalar.activation(dst, po[:], AF.Copy)
                    else:
                        nc.vector.tensor_copy(dst, po[:])

                def emit_out_dma(g, n2):
                    # one descriptor for 4 q-tiles x 512 cols
                    ob4 = ob_group.pop((g, n2))
                    nc.sync.dma_start(
                        out=out_d[g * 512:(g + 1) * 512,
                                  n2 * 512:(n2 + 1) * 512].rearrange(
                                      "(t p) n -> p t n", p=128),
                        in_=ob4[:].rearrange("p (t n) -> p t n", t=4))

                # ---- pre-attention: V' tiles 0-3 hide the xq DMA tail,
                # then the qq0 quarter of the Q projection (st2-tag banks,
                # since psA is closed) ----------------------------------
                for j in range(4):
                    emit_vproj(j)
                pq0_t = psC.tile([128, 1024], F32, tag="st2", bufs=2,
                                 name="pq0")
                pq0 = {p: pq0_t[:, p * 512:(p + 1) * 512] for p in range(2)}
                for d in range(ND):
                    emit_qproj_qc_mm(0, d, pq0)
                emit_qproj_qc_add(0, pq0)

                # filler plans: {block: {j: [thunks]}}
                fillers = {k: {} for k in range(8)}
                # block 0: V' tiles 4-15 at js 0-11, Qproj qc1 at js 12-15
                for j in range(4, NJ):
                    fillers[0][j - 4] = [lambda j=j: emit_vproj(j)]
                pq_w = {}

                def qproj_slot(qc, dd):
                    if dd == 0:
                        pq_w[qc] = {p: psC.tile([128, 512], F32, tag="aux",
                                                bufs=2, name=f"pqw{qc}{p}")[:]
                                    for p in range(2)}
                    emit_qproj_qc_mm(qc, 2 * dd, pq_w[qc])
                    emit_qproj_qc_mm(qc, 2 * dd + 1, pq_w[qc])
                    if dd == 3:
                        emit_qproj_qc_add(qc, pq_w[qc])
                        del pq_w[qc]

                for dd in range(4):
                    fillers[0].setdefault(12 + dd, []).append(
                        lambda dd=dd: qproj_slot(1, dd))
                # blocks 1-2: Qproj qc2/qc3, one d-pair every other j
                for blk, qc in ((1, 2), (2, 3)):
                    for dd in range(4):
                        fillers[blk].setdefault(2 * dd, []).append(
                            lambda qc=qc, dd=dd: qproj_slot(qc, dd))
                # blocks 3/5: output projection of qq0/qq1 at js 2..11
                for blk, qq in ((3, 0), (5, 1)):
                    for n2 in range(2):
                        for i in range(4):
                            fillers[blk].setdefault(2 + n2 * 5 + i, []).append(
                                lambda s=qq * 4 + i, n2=n2:
                                emit_outproj_tile(s, n2))
                        fillers[blk].setdefault(6 + n2 * 5, []).append(
                            lambda g=qq, n2=n2: emit_out_dma(g, n2))
                # block 7: its own AVs ride along (aux o_ps pair)
                for j in range(1, NJ):
                    fillers[7].setdefault(j, []).append(
                        lambda j=j - 1: emit_av(7, j))

                for k in range(8):
                    for j in range(NJ):
                        emit_score_exp(k, j)
                        # AVs of the previous block, one slot behind so the
                        # o_ps pair WAR-wait hides under an exp period
                        if k >= 1 and j >= 1:
                            emit_av(k - 1, j - 1)
                        for thunk in fillers[k].get(j, ()):
                            thunk()
                        if j == 1:
                            flush_norm()
                    if k >= 1:
                        emit_av(k - 1, NJ - 1)
                        emit_norm(k - 1)

                # ---- drain ------------------------------------------------
                # norm chains (DVE/GpSimd) for b6/b7 run concurrently with
                # the qq2 output projection (PE), which only reads o_sbB2
                emit_av(7, NJ - 1)
                emit_norm(7)
                flush_norm()
                for g in (2, 3):
                    for n2 in range(2):
                        for i in range(4):
                            emit_outproj_tile(g * 4 + i, n2, drain=True)
                        emit_out_dma(g, n2)

                if _DBG:
                    nc.sync.dma_start(out=dbg_kt[:], in_=kt_sb[:])
                    nc.sync.dma_start(out=dbg_qt[:], in_=qt_sb[:])
                    nc.sync.dma_start(out=dbg_vp[:], in_=vp_sb[:])
                    nc.sync.dma_start(out=dbg_oA[:], in_=o_sbA[:])
                    for t in range(2):
                        nc.sync.dma_start(
                            out=dbg_oB[:, t * 1024:t * 1024 + 512],
                            in_=o_sbB2[:, t * 512:(t + 1) * 512])
                        nc.sync.dma_start(
                            out=dbg_oB[:, t * 1024 + 512:(t + 1) * 1024],
                            in_=o_sbB3[:, t * 512:(t + 1) * 512])

    nc.compile()
    return nc


def build_in_maps(inputs):
    query_input = np.asarray(inputs["query_input"], dtype=np.float32)
    kv_input = np.asarray(inputs["kv_input"], dtype=np.float32)
    Wq = np.asarray(inputs["Wq"], dtype=np.float32)
    bq = np.asarray(inputs["bq"], dtype=np.float32)
    Wkv = np.asarray(inputs["Wkv"], dtype=np.float32)
    bkv = np.asarray(inputs["bkv"], dtype=np.float32)
    Wo = np.asarray(inputs["Wo"], dtype=np.float32)

    Wk = Wkv[:, :D]
    Wv = Wkv[:, D:]
    bk = bkv[:D]
    ones64 = np.ones((1, 128), np.float32)

    xT = [np.ascontiguousarray(query_input[b].T).astype(BF16NP) for b in range(B)]
    kvT = [np.ascontiguousarray(kv_input[b].T).astype(BF16NP) for b in range(B)]

    in_maps = []
    for c in range(N_CORES):
        b, g = divmod(c, G)
        c0 = g * GC
        wvp = np.zeros((D, HPG * 65), np.float32)
        for h in range(HPG):
                wvp[:, h * 65:h * 65 + 64] = Wv[:, c0 + h * DH:c0 + (h + 1) * DH]
        bq2 = bq[c0:c0 + GC].reshape(2, 128).T.copy()
        bk2 = bk[c0:c0 + GC].reshape(2, 128).T.copy()
        in_maps.append({
                "xqT": xT[b],
                "xkvT": kvT[b],
                "wq": np.ascontiguousarray(Wq[:, c0:c0 + GC]).astype(BF16NP),
                "wk": np.ascontiguousarray(Wk[:, c0:c0 + GC]).astype(BF16NP),
                "wvp": wvp.astype(BF16NP),
                "wo": np.ascontiguousarray(Wo[c0:c0 + GC, :]).astype(BF16NP),
                "bq2": np.ascontiguousarray(bq2),
                "bk2": np.ascontiguousarray(bk2),
                "ones64": ones64,
        })
    return in_maps


def kernel(query_input, kv_input, Wq, bq, Wkv, bkv, Wo, bo):
    global _nc_cache
    from concourse import bass_utils

    if _nc_cache is None:
        _nc_cache = _build_nc()
    nc = _nc_cache

    Wkv = np.asarray(Wkv, dtype=np.float32)
    Wo = np.asarray(Wo, dtype=np.float32)
    bo = np.asarray(bo, dtype=np.float32)
    bv = np.asarray(bkv, np.float32)[D:]

    in_maps = build_in_maps(dict(
        query_input=query_input, kv_input=kv_input, Wq=Wq, bq=bq,
        Wkv=Wkv, bkv=bkv, Wo=Wo))

    res = bass_utils.run_bass_kernel_spmd(nc, in_maps,
                                          core_ids=list(range(N_CORES)))

    # gather: sum the 4 head-group partials per batch; add biases the device
    # left out (bo, and bv which passes through Wo since softmax rows sum to 1)
    tail = bv @ Wo + bo
    out = np.empty((B, SQ, D), np.float32)
    for b in range(B):
        acc = res.results[b * G + 0]["out_p"].astype(np.float32).copy()
        for g in range(1, G):
                acc += res.results[b * G + g]["out_p"]
        out[b] = acc + tail[None, :]
    return out
